# revision 20
# baseline (speedup 1.0000x reference)
"""Two-layer GAT on 8 Trainium2 NeuronCores.

Strategy (dst-sharded, node-major bf16 table):
 - Nodes are degree-sorted into 128-node blocks; blocks are dealt round-robin
   to the 8 cores so every core runs an identical static schedule. Rounds are
   grouped GL=7 at a time; one PSUM bank accumulates all 7 rounds (7*65=455
   cols <= 512).
 - Per layer, each core computes its shard of a node table
   [row: h(64 bf16) | asrc(f32) | adst(f32) | pad -> 128 bf16 = 256B], an
   AllGather replicates the full table, then each (group, bank) issues ONE
   dma_gather whose index order interleaves the group's rounds
   (block = slot_d * gl + round_local), so slot-d of all 7 rounds is one
   matmul rhs [128, gl, 65] accumulating into po[128, gl, 65].
 - t_e = exp(leaky_relu(asrc_src + adst_dst)) is computed on the gathered
   chunk in a handful of wide vector ops; t overwrites the asrc slot (bf16)
   so rhs cols 0:65 are [t*h | t] after one broadcast multiply.
 - Softmax max-subtraction is algebraically a no-op here (scores are O(10)).
   Padded slots gather a dummy table row with asrc = -1e30 so t == 0.
 - Two int16-index banks (A at row baseA, B at baseB) cover the >64K rows.
"""
import numpy as np

_CACHE = {}


def _host_prep(x, edge_index, cfg):
    N, C, R, GL = cfg["N"], 8, cfg["R"], cfg["GL"]
    NPC = R * 128            # rows per core shard
    NTOT = C * NPC
    RH = (R + 1) // 2        # rounds in table half 0 (AllGather split point)
    baseA, baseB, span = cfg["baseA"], cfg["baseB"], cfg["span"]
    A_hi = min(NTOT - 1, baseA + span)
    B_lo = max(0, baseB - span)
    assert A_hi >= B_lo - 1

    groups = [(i * GL, min(GL, R - i * GL)) for i in range((R + GL - 1) // GL)]
    NG = len(groups)

    src = np.asarray(edge_index[0], dtype=np.int64)
    dst = np.asarray(edge_index[1], dtype=np.int64)
    E = src.shape[0]

    deg = np.bincount(dst, minlength=N)
    odeg = np.bincount(src, minlength=N)
    order = np.argsort(-deg, kind="stable")
    all_nodes = np.concatenate([order, np.full(NTOT - N, -1, dtype=np.int64)])

    m = np.arange(NTOT)
    b = m // 128
    p = m % 128
    rnd = b // C
    core = b % C
    # half-major row layout so each AllGather half is a contiguous table range
    half = rnd // RH
    rw = rnd % RH
    row_of_listpos = (half * (C * RH * 128) + core * (RH * 128) + 128 * rw + p)

    # within each round, put the highest OUT-degree nodes on rows inside the
    # flex window [B_lo, A_hi] — their out-edges become bank-flexible, which
    # shrinks the forced-bank count tails that drive slot padding
    for r in range(R):
        sel = np.flatnonzero(rnd == r)
        rows = row_of_listpos[sel]
        flex = (rows >= B_lo) & (rows <= A_hi)
        nd = all_nodes[sel]
        od = np.where(nd >= 0, odeg[np.clip(nd, 0, None)], -1)
        pos_order = np.argsort(~flex, kind="stable")   # flex positions first
        nd_order = np.argsort(-od, kind="stable")      # high out-degree first
        newnd = np.empty_like(nd)
        newnd[pos_order] = nd[nd_order]
        all_nodes[sel] = newnd

    # bank holes + dummy rows must hold pad nodes
    special_rows = {baseA - 1, baseA, baseB - 1, baseB}
    row_to_listpos = np.empty(NTOT, dtype=np.int64)
    row_to_listpos[row_of_listpos] = m
    pad_positions = [i for i in range(NTOT - 1, -1, -1) if all_nodes[i] < 0]
    pi = 0
    for r in special_rows:
        lp = row_to_listpos[r]
        if all_nodes[lp] >= 0:
            while pi < len(pad_positions):
                q = pad_positions[pi]; pi += 1
                if row_of_listpos[q] not in special_rows and all_nodes[q] < 0:
                    all_nodes[lp], all_nodes[q] = all_nodes[q], all_nodes[lp]
                    break

    node_at_listpos = all_nodes
    row_of_node = np.full(N, -1, dtype=np.int64)
    real = node_at_listpos >= 0
    row_of_node[node_at_listpos[real]] = row_of_listpos[real]

    sr = row_of_node[src]
    dr = row_of_node[dst]

    rnd_of_node_row = np.empty(NTOT, dtype=np.int64)
    rnd_of_node_row[row_of_listpos] = rnd

    # bank per edge (0=A, 1=B); per-group thresholds TA/TB minimize the
    # rectangular slot count max(cntA) + max(cntB) over the group
    canA = sr <= A_hi
    canB = sr >= B_lo
    forcedA = canA & ~canB
    forcedB = ~canA & canB
    flex = canA & canB
    nA0 = np.bincount(dr[forcedA], minlength=NTOT)
    nB0 = np.bincount(dr[forcedB], minlength=NTOT)
    nf = np.bincount(dr[flex], minlength=NTOT)
    degr = np.bincount(dr, minlength=NTOT)

    # per-round thresholds TA/TB minimize the per-round slot count
    # max(cntA) + max(cntB) (jagged layout bills each round individually)
    A0r_ = np.zeros(R, dtype=np.int64)
    B0r_ = np.zeros(R, dtype=np.int64)
    Mr_ = np.zeros(R, dtype=np.int64)
    np.maximum.at(A0r_, rnd_of_node_row, nA0)
    np.maximum.at(B0r_, rnd_of_node_row, nB0)
    np.maximum.at(Mr_, rnd_of_node_row, degr)
    costr = np.maximum(Mr_, A0r_ + B0r_)
    TAr = np.clip((costr + 1) // 2, A0r_, costr - B0r_)
    TBr = costr - TAr
    r_of_row = rnd_of_node_row
    lo = np.maximum(nA0, degr - TBr[r_of_row])
    hi = np.minimum(TAr[r_of_row], nA0 + nf)
    cntA = np.clip((degr + 1) // 2, lo, hi)

    o = np.argsort(dr[flex], kind="stable")
    flex_idx = np.nonzero(flex)[0][o]
    grp = dr[flex_idx]
    uniq, first = np.unique(grp, return_index=True)
    fr = np.arange(len(grp)) - first[np.searchsorted(uniq, grp)]
    bank = np.ones(E, dtype=np.int8)
    bank[forcedA] = 0
    bank[flex_idx] = (fr >= (cntA[grp] - nA0[grp])).astype(np.int8)
    cntB = degr - cntA

    DAr = np.zeros(R, dtype=np.int64)
    DBr = np.zeros(R, dtype=np.int64)
    np.maximum.at(DAr, rnd_of_node_row, cntA)
    np.maximum.at(DBr, rnd_of_node_row, cntB)
    # rect-A slot d=0 must exist in every round: the group's first matmul
    # (bank A, d=0) covers all po columns with start=True
    DAr = np.maximum(DAr, 1)

    # slot position within (dst, bank); negative gather indices first so the
    # final slot of each (dst, bank) sequence is non-negative where possible
    idxval = np.where(bank == 0, sr - baseA, sr - baseB)
    nonneg = (idxval >= 0).astype(np.int8)
    o2 = np.lexsort((nonneg, bank, dr))
    grp2 = dr[o2] * 2 + bank[o2]
    uniq2, first2 = np.unique(grp2, return_index=True)
    dpos = np.arange(E) - first2[np.searchsorted(uniq2, grp2)]
    d_of_edge = np.empty(E, dtype=np.int64)
    d_of_edge[o2] = dpos

    cnt_nonneg_A = np.bincount(dr[(bank == 0) & (idxval >= 0)], minlength=NTOT)
    cnt_nonneg_B = np.bincount(dr[(bank == 1) & (idxval >= 0)], minlength=NTOT)
    p_of_row = np.empty(NTOT, dtype=np.int64)
    p_of_row[row_of_listpos] = p
    is_last_p = p_of_row == 127

    def build_plan(Dr, cnt, cnt_nonneg):
        # jagged layout per (group, bank): rect core (d < dmin over the
        # group's rounds) + per-d tail runs of rounds still alive. Returns
        # (plans, offsets); bumps Dr where the call's final slot would be a
        # full all-negative (dst,bank) sequence (HW drops trailing negatives).
        while True:
            plans = []
            off = [0]
            redo = False
            for g, (g0, gl) in enumerate(groups):
                D = Dr[g0:g0 + gl]
                dmin = int(D.min())
                S = dmin * gl
                runs = []
                last_rl = gl - 1
                for d in range(dmin, int(D.max())):
                    rl = 0
                    while rl < gl:
                        if D[rl] > d:
                            rl0 = rl
                            while rl < gl and D[rl] > d:
                                rl += 1
                            runs.append((d, rl0, rl - rl0, S))
                            S += rl - rl0
                            last_rl = rl - 1
                        else:
                            rl += 1
                plans.append(dict(dmin=dmin, S=S, runs=runs))
                off.append(off[-1] + S)
                # guard: dst at (round of final block, p=127) must not have a
                # full all-negative slot sequence
                rr = g0 + last_rl
                sel = is_last_p & (rnd_of_node_row == rr)
                if np.any(sel & (cnt == Dr[rr]) & (cnt > 0) & (cnt_nonneg == 0)):
                    Dr[rr] += 1
                    redo = True
                    break
            if not redo:
                return plans, off

    planA, offA = build_plan(DAr, cntA, cnt_nonneg_A)
    planB, offB = build_plan(DBr, cntB, cnt_nonneg_B)
    SA, SB = int(offA[-1]), int(offB[-1])

    def blk_lut(Dr, plans, off):
        maxD = max(int(Dr.max()), 1)
        lut = np.full((R, maxD), -1, dtype=np.int64)
        for g, (g0, gl) in enumerate(groups):
            pl = plans[g]
            dmin = pl["dmin"]
            for rl in range(gl):
                for d in range(dmin):
                    lut[g0 + rl, d] = off[g] + d * gl + rl
            for (d, rl0, n, blk0) in pl["runs"]:
                for j in range(n):
                    lut[g0 + rl0 + j, d] = off[g] + blk0 + j
        return lut

    lutA = blk_lut(DAr, planA, offA)
    lutB = blk_lut(DBr, planB, offB)

    idxA = np.zeros((C, SA * 128), dtype=np.int32)
    idxB = np.zeros((C, SB * 128), dtype=np.int32)
    e_half = dr // (C * RH * 128)
    e_rem = dr % (C * RH * 128)
    e_core = e_rem // (RH * 128)
    e_rnd = e_half * RH + (e_rem % (RH * 128)) // 128
    e_p = dr % 128
    isA = bank == 0
    blkA = lutA[e_rnd[isA], d_of_edge[isA]]
    assert (blkA >= 0).all()
    idxA[e_core[isA], blkA * 128 + e_p[isA]] = sr[isA] - baseA
    isB = ~isA
    blkB = lutB[e_rnd[isB], d_of_edge[isB]]
    assert (blkB >= 0).all()
    idxB[e_core[isB], blkB * 128 + e_p[isB]] = sr[isB] - baseB
    assert idxA.min() >= -32768 and idxA.max() <= 32766
    assert idxB.min() >= -32768 and idxB.max() <= 32766
    # final slot of every call must be non-negative (trailing negatives drop)
    for g in range(NG):
        if offA[g + 1] > offA[g]:
            assert (idxA[:, offA[g + 1] * 128 - 1] >= 0).all()
        if offB[g + 1] > offB[g]:
            assert (idxB[:, offB[g + 1] * 128 - 1] >= 0).all()

    def wrap(a):  # [C, S*128] -> [C, 128, S*8] int16 (16-wrap, replicated x8)
        Cn, tot = a.shape
        if tot == 0:
            return np.zeros((Cn, 128, 0), dtype=np.int16)
        w = a.reshape(Cn, tot // 16, 16).transpose(0, 2, 1)
        return np.ascontiguousarray(np.tile(w, (1, 8, 1))).astype(np.int16)

    # shard-local position (round-major) differs from the half-major table row
    shardpos = 128 * rnd + p
    xT = np.zeros((C, x.shape[1], NPC), dtype=np.float32)
    xf = np.asarray(x, dtype=np.float32)
    for k in range(C):
        sel = (core == k) & real
        xT[k][:, shardpos[sel]] = xf[node_at_listpos[sel]].T

    # host-side gather index: node -> (core, shardpos) in concatenated output
    outpos_of_listpos = core * NPC + shardpos
    outpos_of_node = np.full(N, -1, dtype=np.int64)
    outpos_of_node[node_at_listpos[real]] = outpos_of_listpos[real]

    return dict(
        idxA=wrap(idxA), idxB=wrap(idxB), xT=xT,
        planA=planA, planB=planB,
        offA=[int(v) for v in offA], offB=[int(v) for v in offB],
        groups=groups, SA=SA, SB=SB, RH=RH,
        row_of_node=outpos_of_node, table_row_of_node=row_of_node,
    )


def _plan_key(plans):
    return tuple((p["dmin"], p["S"], tuple(p["runs"])) for p in plans)


def _build(cfg, planA, planB, offA, offB, groups, SA, SB, RH):
    import sys
    if "/opt/trn_rl_repo" not in sys.path:
        sys.path.insert(0, "/opt/trn_rl_repo")
    import concourse.mybir as mybir
    import concourse.tile as tile
    from concourse import bacc
    from concourse.masks import make_identity

    f32 = mybir.dt.float32
    bf16 = mybir.dt.bfloat16
    R, GL = cfg["R"], cfg["GL"]
    F, HD = cfg["F"], cfg["H"]
    NPC = R * 128
    NTOT = 8 * NPC
    baseA, baseB = cfg["baseA"], cfg["baseB"]
    AF = HD + 2  # h | asrc | adst (f32 table-build layout)
    NG = len(groups)
    NPCa = RH * 128          # shard half sizes (round-major)
    NPCb = NPC - NPCa

    nc = bacc.Bacc("TRN2", target_bir_lowering=False, debug=False, num_devices=8)
    xT_t = nc.dram_tensor("xT", [F, NPC], f32, kind="ExternalInput")
    iA_t = nc.dram_tensor("idxA", [128, SA * 8], mybir.dt.int16, kind="ExternalInput")
    iB_t = nc.dram_tensor("idxB", [128, SB * 8], mybir.dt.int16, kind="ExternalInput")
    W1_t = nc.dram_tensor("W1", [F, HD], f32, kind="ExternalInput")
    W2_t = nc.dram_tensor("W2", [HD, HD], f32, kind="ExternalInput")
    av_t = nc.dram_tensor("avec", [4, HD], f32, kind="ExternalInput")
    bv_t = nc.dram_tensor("bvec", [2, HD], f32, kind="ExternalInput")
    out_t = nc.dram_tensor("out", [NPC, HD], f32, kind="ExternalOutput")

    # shard halves are separate tensors so the AllGather of half a never
    # false-depends on phase-A writes of half b
    shard1a = nc.dram_tensor("shard1a", [NPCa, 128], bf16, kind="Internal")
    shard1b = nc.dram_tensor("shard1b", [NPCb, 128], bf16, kind="Internal")
    shard2a = nc.dram_tensor("shard2a", [NPCa, 128], bf16, kind="Internal")
    shard2b = nc.dram_tensor("shard2b", [NPCb, 128], bf16, kind="Internal")
    table1 = nc.dram_tensor("table1", [NTOT, 128], bf16, kind="Internal",
                            addr_space="Shared")
    table2 = nc.dram_tensor("table2", [NTOT, 128], bf16, kind="Internal",
                            addr_space="Shared")
    shards = {(1, 0): shard1a, (1, 1): shard1b, (2, 0): shard2a, (2, 1): shard2b}

    RG = [[0, 1, 2, 3, 4, 5, 6, 7]]

    with tile.TileContext(nc) as tc:
        with tc.tile_pool(name="const", bufs=1) as cp, \
             tc.tile_pool(name="spool", bufs=3) as sp, \
             tc.tile_pool(name="gpool", bufs=3) as gp, \
             tc.tile_pool(name="mpool", bufs=2) as mp, \
             tc.tile_pool(name="hpool", bufs=NG) as hp, \
             tc.tile_pool(name="psA", bufs=2, space="PSUM") as psA, \
             tc.tile_pool(name="psT", bufs=2, space="PSUM") as psT, \
             tc.tile_pool(name="psO", bufs=2, space="PSUM") as psO:

            ident = cp.tile([128, 128], f32)
            make_identity(nc, ident[:])
            identb = cp.tile([128, 128], bf16)
            make_identity(nc, identb[:])

            # weight prep: aug[l] = [W | W@a_src | W@a_dst]  ([K, AF])
            augs = []
            for l, (Wt, K) in enumerate(((W1_t, F), (W2_t, HD))):
                Wsb = cp.tile([K, HD], f32, tag=f"w{l}")
                nc.sync.dma_start(out=Wsb[:], in_=Wt.ap()[:, :])
                Wt_ps = psT.tile([HD, K], f32, tag="pst")
                nc.tensor.transpose(out=Wt_ps[:], in_=Wsb[:], identity=ident[:K, :K])
                Wtr = cp.tile([HD, K], f32, tag=f"wt{l}")
                nc.vector.tensor_copy(out=Wtr[:], in_=Wt_ps[:])
                aug = cp.tile([K, AF], f32, tag=f"aug{l}")
                nc.vector.tensor_copy(out=aug[:, 0:HD], in_=Wsb[:])
                for s in range(2):
                    acol = cp.tile([HD, 1], f32, tag=f"ac{l}{s}")
                    nc.sync.dma_start(
                        out=acol[:],
                        in_=av_t.ap()[2 * l + s:2 * l + s + 1, :].rearrange("a b -> b a"))
                    wa_ps = psT.tile([K, 1], f32, tag="pst")
                    nc.tensor.matmul(out=wa_ps[:], lhsT=Wtr[:], rhs=acol[:],
                                     start=True, stop=True)
                    nc.vector.tensor_copy(out=aug[:, HD + s:HD + s + 1], in_=wa_ps[:])
                augs.append(aug)

            # bias, replicated GL times: bbg[l] = [128, GL, HD]
            bbg = []
            for l in range(2):
                t = cp.tile([128, HD], f32, tag=f"b{l}")
                nc.sync.dma_start(out=t[:1, :], in_=bv_t.ap()[l:l + 1, :])
                nc.gpsimd.partition_broadcast(t[:], t[:1, :])
                tg = cp.tile([128, GL, HD], f32, tag=f"bg{l}")
                for rl in range(GL):
                    nc.vector.tensor_copy(out=tg[:, rl, :], in_=t[:])
                bbg.append(tg)

            # dummy row: h = 0, asrc = -1e30 (f32 at bf16 cols 64:66)
            dumrow = cp.tile([1, 128], bf16)
            nc.vector.memset(dumrow[:], 0.0)
            nc.vector.memset(dumrow[:, 64:66].bitcast(f32), -1e30)

            iA_sb = cp.tile([128, SA * 8], mybir.dt.int16)
            nc.sync.dma_start(out=iA_sb[:], in_=iA_t.ap()[:, :])
            iB_sb = cp.tile([128, SB * 8], mybir.dt.int16)
            nc.sync.dma_start(out=iB_sb[:], in_=iB_t.ap()[:, :])

            adst_own1 = cp.tile([128, R], f32, tag="adst1")
            adst_own2 = cp.tile([128, R], f32, tag="adst2")
            adst_own = [adst_own1, adst_own2]

            def table_chunk_write(i, hs, lnum, layer):
                # hs: SBUF [AF, 128] f-major -> bf16 node-major rows
                htp = psT.tile([128, AF], f32, tag="pst")
                nc.tensor.transpose(out=htp[:], in_=hs[:], identity=ident[:AF, :AF])
                chunk = sp.tile([128, 128], bf16, tag="chunk")
                nc.vector.tensor_copy(out=chunk[:, 0:HD], in_=htp[:, 0:HD])
                nc.vector.tensor_copy(out=chunk[:, 64:66].bitcast(f32),
                                      in_=htp[:, HD:HD + 1])
                nc.vector.tensor_copy(out=chunk[:, 66:68].bitcast(f32),
                                      in_=htp[:, HD + 1:HD + 2])
                nc.vector.tensor_copy(out=adst_own[layer][:, i:i + 1],
                                      in_=htp[:, HD + 1:HD + 2])
                hf, ro = (0, i) if i < RH else (1, i - RH)
                nc.sync.dma_start(
                    out=shards[(lnum, hf)].ap()[128 * ro:128 * (ro + 1), :],
                    in_=chunk[:])

            def allgather_half(lnum, table, hf):
                shard = shards[(lnum, hf)]
                n_in = NPCa if hf == 0 else NPCb
                o0 = 0 if hf == 0 else 8 * NPCa
                nc.gpsimd.collective_compute(
                    "AllGather", mybir.AluOpType.bypass, RG,
                    ins=[shard.ap()[:, :]],
                    outs=[table.ap()[o0:o0 + 8 * n_in, :]])
                for base in (baseA, baseB):
                    if o0 <= base < o0 + 8 * n_in:
                        nc.gpsimd.dma_start(out=table.ap()[base:base + 1, :],
                                            in_=dumrow[:])

            def phase_A1():
                for t in range(R):
                    rhs = sp.tile([F, 128], f32, tag="parhs")
                    nc.sync.dma_start(out=rhs[:], in_=xT_t.ap()[:, 128 * t:128 * (t + 1)])
                    hp_ = psA.tile([AF, 128], f32, tag="paps")
                    nc.tensor.matmul(out=hp_[:], lhsT=augs[0][:], rhs=rhs[:],
                                     start=True, stop=True)
                    hs = sp.tile([AF, 128], f32, tag="pahs")
                    nc.scalar.copy(out=hs[:], in_=hp_[:])
                    table_chunk_write(t, hs, 1, 0)
                    if t == RH - 1:
                        allgather_half(1, table1, 0)
                allgather_half(1, table1, 1)

            def phase_B_group(layer, g, table, adst):
                final = layer == 1
                g0, gl = groups[g]
                plA, plB = planA[g], planB[g]
                BA, BB = plA["S"], plB["S"]
                GA = gp.tile([128, max(BA, 1), 128], bf16, tag="G")
                if BA:
                    nc.gpsimd.dma_gather(
                        out_ap=GA[:, 0:BA, :], in_ap=table.ap()[baseA:, :],
                        idxs_ap=iA_sb[:, offA[g] * 8:offA[g + 1] * 8],
                        num_idxs=128 * BA, num_idxs_reg=128 * BA,
                        elem_size=128, single_packet=False)
                GB = gp.tile([128, max(BB, 1), 128], bf16, tag="G")
                if BB:
                    nc.gpsimd.dma_gather(
                        out_ap=GB[:, 0:BB, :], in_ap=table.ap()[baseB:, :],
                        idxs_ap=iB_sb[:, offB[g] * 8:offB[g + 1] * 8],
                        num_idxs=128 * BB, num_idxs_reg=128 * BB,
                        elem_size=128, single_packet=False)

                po = psO.tile([128, gl, HD + 1], f32, tag="po")
                nmm = sum(pl["dmin"] + len(pl["runs"])
                          for pl, B in ((plA, BA), (plB, BB)) if B)
                mm_i = 0
                for (G, pl, B, btag) in ((GA, plA, BA, "a"), (GB, plB, BB, "b")):
                    if B == 0:
                        continue
                    dmin, runs = pl["dmin"], pl["runs"]
                    # scores on the asrc (f32) subfield; adst replicated per
                    # round across the interleaved rect blocks + tail runs
                    arep = mp.tile([128, B, 1], f32, tag="arep" + btag)
                    if dmin:
                        ar4 = arep[:, 0:dmin * gl, :].rearrange(
                            "p (d r) o -> p d r o", d=dmin)
                        for rl in range(gl):
                            nc.vector.tensor_copy(
                                out=ar4[:, 0:dmin, rl, 0],
                                in_=adst[:, g0 + rl:g0 + rl + 1].to_broadcast(
                                    [128, dmin]))
                    for (d, rl0, n, blk0) in runs:
                        nc.vector.tensor_copy(
                            out=arep[:, blk0:blk0 + n, 0],
                            in_=adst[:, g0 + rl0:g0 + rl0 + n])
                    zt = mp.tile([128, B, 1], f32, tag="zt" + btag)
                    nc.vector.tensor_tensor(
                        out=zt[:, 0:B, :], in0=G[:, 0:B, 64:66].bitcast(f32),
                        in1=arep[:, 0:B, :], op=mybir.AluOpType.add)
                    z2 = mp.tile([128, B, 1], f32, tag="z2" + btag)
                    nc.vector.tensor_scalar(
                        out=z2[:, 0:B, :], in0=zt[:, 0:B, :],
                        scalar1=cfg["slope"], scalar2=None,
                        op0=mybir.AluOpType.mult)
                    lt = mp.tile([128, B, 1], f32, tag="lt" + btag)
                    nc.vector.tensor_tensor(
                        out=lt[:, 0:B, :], in0=zt[:, 0:B, :], in1=z2[:, 0:B, :],
                        op=mybir.AluOpType.max)
                    # t (bf16) overwrites the asrc-lo slot -> col 64
                    nc.scalar.activation(
                        out=G[:, 0:B, 64:65], in_=lt[:, 0:B, :],
                        func=mybir.ActivationFunctionType.Exp)
                    # weighted messages in place: h *= t
                    nc.vector.tensor_tensor(
                        out=G[:, 0:B, 0:HD], in0=G[:, 0:B, 0:HD],
                        in1=G[:, 0:B, 64:65].to_broadcast([128, B, HD]),
                        op=mybir.AluOpType.mult)
                    if dmin:
                        G4 = G[:, 0:dmin * gl, 0:HD + 1].rearrange(
                            "p (d r) f -> p d r f", d=dmin)
                        for d in range(dmin):
                            nc.tensor.matmul(
                                out=po[:, 0:gl, :], lhsT=identb[:],
                                rhs=G4[:, d, :, :],
                                start=mm_i == 0, stop=mm_i == nmm - 1)
                            mm_i += 1
                    for (d, rl0, n, blk0) in runs:
                        nc.tensor.matmul(
                            out=po[:, rl0:rl0 + n, :], lhsT=identb[:],
                            rhs=G[:, blk0:blk0 + n, 0:HD + 1],
                            start=mm_i == 0, stop=mm_i == nmm - 1,
                            skip_group_check=True)
                        mm_i += 1

                den = mp.tile([128, gl, 1], f32, tag="den")
                nc.vector.tensor_scalar_max(out=den[:, 0:gl, :],
                                            in0=po[:, 0:gl, HD:HD + 1],
                                            scalar1=1e-16)
                rd = mp.tile([128, gl, 1], f32, tag="rd")
                nc.vector.reciprocal(out=rd[:, 0:gl, :], in_=den[:, 0:gl, :])
                h = (mp if final else hp).tile([128, gl, HD], f32,
                                               tag="hfin" + str(layer))
                nc.vector.tensor_tensor(
                    out=h[:, 0:gl, :], in0=po[:, 0:gl, 0:HD],
                    in1=rd[:, 0:gl, :].to_broadcast([128, gl, HD]),
                    op=mybir.AluOpType.mult)
                nc.vector.tensor_tensor(out=h[:, 0:gl, :], in0=h[:, 0:gl, :],
                                        in1=bbg[layer][:, 0:gl, :],
                                        op=mybir.AluOpType.add)
                if final:
                    nc.sync.dma_start(
                        out=out_t.ap()[128 * g0:128 * (g0 + gl), :].rearrange(
                            "(r p) f -> p r f", r=gl),
                        in_=h[:, 0:gl, :])
                else:
                    nc.scalar.activation(out=h[:, 0:gl, :], in_=h[:, 0:gl, :],
                                         func=mybir.ActivationFunctionType.Relu)
                    hkeep.append(h)

            def phase_A2_group(g):
                g0, gl = groups[g]
                h = hkeep[g]
                for rl in range(gl):
                    t = g0 + rl
                    htr = psT.tile([HD, 128], f32, tag="pst")
                    nc.tensor.transpose(out=htr[:], in_=h[:, rl, :],
                                        identity=ident[:])
                    ht = sp.tile([HD, 128], f32, tag="hTs")
                    nc.scalar.copy(out=ht[:], in_=htr[:])
                    hp2 = psA.tile([AF, 128], f32, tag="paps")
                    nc.tensor.matmul(out=hp2[:], lhsT=augs[1][:], rhs=ht[:],
                                     start=True, stop=True)
                    hs2 = sp.tile([AF, 128], f32, tag="pahs")
                    nc.scalar.copy(out=hs2[:], in_=hp2[:])
                    table_chunk_write(t, hs2, 2, 1)

            hkeep = []
            phase_A1()
            ag2a_done = False
            for g in range(NG):
                phase_B_group(0, g, table1, adst_own[0])
                phase_A2_group(g)
                g0, gl = groups[g]
                if not ag2a_done and g0 + gl >= RH:
                    allgather_half(2, table2, 0)
                    ag2a_done = True
            allgather_half(2, table2, 1)
            for g in range(NG):
                phase_B_group(1, g, table2, adst_own[1])

    nc.compile()
    return nc


def _make_cfg(N, F, H):
    if N >= 32768:
        return dict(N=N, R=98, GL=7, baseA=32768, baseB=67585, span=32766,
                    F=F, H=H, slope=0.2)
    NTOT = max(2048, ((N + 128 + 1023) // 1024) * 1024)
    R = NTOT // 1024
    return dict(N=N, R=R, GL=min(7, R), baseA=NTOT // 4, baseB=(3 * NTOT) // 4,
                span=min(32766, (5 * NTOT) // 8), F=F, H=H, slope=0.2)


def _make_in_maps(inputs, prep):
    avec = np.stack([np.asarray(inputs["a1_src"]), np.asarray(inputs["a1_dst"]),
                     np.asarray(inputs["a2_src"]), np.asarray(inputs["a2_dst"])]
                    ).astype(np.float32)
    bvec = np.stack([np.asarray(inputs["b1"]), np.asarray(inputs["b2"])]
                    ).astype(np.float32)
    in_maps = []
    for k in range(8):
        in_maps.append({
            "xT": prep["xT"][k], "idxA": prep["idxA"][k], "idxB": prep["idxB"][k],
            "W1": np.asarray(inputs["W1"], dtype=np.float32),
            "W2": np.asarray(inputs["W2"], dtype=np.float32),
            "avec": avec, "bvec": bvec,
        })
    return in_maps


def kernel(x, edge_index, W1, a1_src, a1_dst, b1, W2, a2_src, a2_dst, b2):
    import sys
    if "/opt/trn_rl_repo" not in sys.path:
        sys.path.insert(0, "/opt/trn_rl_repo")
    from concourse import bass_utils

    x = np.asarray(x)
    cfg = _make_cfg(x.shape[0], x.shape[1], np.asarray(W1).shape[1])
    prep = _host_prep(x, edge_index, cfg)
    key = (cfg["N"], cfg["R"], prep["SA"], prep["SB"],
           _plan_key(prep["planA"]), _plan_key(prep["planB"]))
    if key not in _CACHE:
        _CACHE[key] = _build(cfg, prep["planA"], prep["planB"],
                             prep["offA"], prep["offB"], prep["groups"],
                             prep["SA"], prep["SB"], prep["RH"])
    nc = _CACHE[key]

    in_maps = _make_in_maps(
        dict(a1_src=a1_src, a1_dst=a1_dst, a2_src=a2_src, a2_dst=a2_dst,
             b1=b1, b2=b2, W1=W1, W2=W2), prep)
    res = bass_utils.run_bass_kernel_spmd(nc, in_maps, core_ids=list(range(8)))
    shards = np.concatenate([res.results[k]["out"] for k in range(8)], axis=0)
    return shards[prep["row_of_node"]].astype(np.float32)


# revision 24
# speedup vs baseline: 1.0096x; 1.0096x over previous
"""Two-layer GAT on 8 Trainium2 NeuronCores.

Strategy (dst-sharded, node-major bf16 table):
 - Nodes are degree-sorted into 128-node blocks; blocks are dealt round-robin
   to the 8 cores so every core runs an identical static schedule. Rounds are
   grouped GL=7 at a time; one PSUM bank accumulates all 7 rounds (7*65=455
   cols <= 512).
 - Per layer, each core computes its shard of a node table
   [row: h(64 bf16) | asrc(f32) | adst(f32) | pad -> 128 bf16 = 256B], an
   AllGather replicates the full table, then each (group, bank) issues ONE
   dma_gather whose index order interleaves the group's rounds
   (block = slot_d * gl + round_local), so slot-d of all 7 rounds is one
   matmul rhs [128, gl, 65] accumulating into po[128, gl, 65].
 - t_e = exp(leaky_relu(asrc_src + adst_dst)) is computed on the gathered
   chunk in a handful of wide vector ops; t overwrites the asrc slot (bf16)
   so rhs cols 0:65 are [t*h | t] after one broadcast multiply.
 - Softmax max-subtraction is algebraically a no-op here (scores are O(10)).
   Padded slots gather a dummy table row with asrc = -1e30 so t == 0.
 - Two int16-index banks (A at row baseA, B at baseB) cover the >64K rows.
"""
import numpy as np

_CACHE = {}


def _host_prep(x, edge_index, cfg):
    N, C, R, GL = cfg["N"], 8, cfg["R"], cfg["GL"]
    NPC = R * 128            # rows per core shard
    NTOT = C * NPC
    # rounds in table half 0 (AllGather split point); RH == R disables split
    # (shared-output AllGather makes the split a wash on HW, and fewer/bigger
    # collectives are strictly better in the cost model)
    RH = (R + 1) // 2 if cfg.get("agsplit", False) else R
    baseA, baseB, span = cfg["baseA"], cfg["baseB"], cfg["span"]
    A_hi = min(NTOT - 1, baseA + span)
    B_lo = max(0, baseB - span)
    assert A_hi >= B_lo - 1

    groups = [(i * GL, min(GL, R - i * GL)) for i in range((R + GL - 1) // GL)]
    NG = len(groups)

    src = np.asarray(edge_index[0], dtype=np.int64)
    dst = np.asarray(edge_index[1], dtype=np.int64)
    E = src.shape[0]

    deg = np.bincount(dst, minlength=N)
    odeg = np.bincount(src, minlength=N)
    order = np.argsort(-deg, kind="stable")
    all_nodes = np.concatenate([order, np.full(NTOT - N, -1, dtype=np.int64)])

    m = np.arange(NTOT)
    b = m // 128
    p = m % 128
    rnd = b // C
    core = b % C
    # half-major row layout so each AllGather half is a contiguous table range
    half = rnd // RH
    rw = rnd % RH
    row_of_listpos = (half * (C * RH * 128) + core * (RH * 128) + 128 * rw + p)

    # within each round, put the highest OUT-degree nodes on rows inside the
    # flex window [B_lo, A_hi] — their out-edges become bank-flexible, which
    # shrinks the forced-bank count tails that drive slot padding
    for r in range(R):
        sel = np.flatnonzero(rnd == r)
        rows = row_of_listpos[sel]
        flex = (rows >= B_lo) & (rows <= A_hi)
        nd = all_nodes[sel]
        od = np.where(nd >= 0, odeg[np.clip(nd, 0, None)], -1)
        pos_order = np.argsort(~flex, kind="stable")   # flex positions first
        nd_order = np.argsort(-od, kind="stable")      # high out-degree first
        newnd = np.empty_like(nd)
        newnd[pos_order] = nd[nd_order]
        all_nodes[sel] = newnd

    # bank holes + dummy rows must hold pad nodes
    special_rows = {baseA - 1, baseA, baseB - 1, baseB}
    row_to_listpos = np.empty(NTOT, dtype=np.int64)
    row_to_listpos[row_of_listpos] = m
    pad_positions = [i for i in range(NTOT - 1, -1, -1) if all_nodes[i] < 0]
    pi = 0
    for r in special_rows:
        lp = row_to_listpos[r]
        if all_nodes[lp] >= 0:
            while pi < len(pad_positions):
                q = pad_positions[pi]; pi += 1
                if row_of_listpos[q] not in special_rows and all_nodes[q] < 0:
                    all_nodes[lp], all_nodes[q] = all_nodes[q], all_nodes[lp]
                    break

    node_at_listpos = all_nodes
    row_of_node = np.full(N, -1, dtype=np.int64)
    real = node_at_listpos >= 0
    row_of_node[node_at_listpos[real]] = row_of_listpos[real]

    sr = row_of_node[src]
    dr = row_of_node[dst]

    rnd_of_node_row = np.empty(NTOT, dtype=np.int64)
    rnd_of_node_row[row_of_listpos] = rnd

    # bank per edge (0=A, 1=B); per-group thresholds TA/TB minimize the
    # rectangular slot count max(cntA) + max(cntB) over the group
    canA = sr <= A_hi
    canB = sr >= B_lo
    forcedA = canA & ~canB
    forcedB = ~canA & canB
    flex = canA & canB
    nA0 = np.bincount(dr[forcedA], minlength=NTOT)
    nB0 = np.bincount(dr[forcedB], minlength=NTOT)
    nf = np.bincount(dr[flex], minlength=NTOT)
    degr = np.bincount(dr, minlength=NTOT)

    # per-round thresholds TA/TB minimize the per-round slot count
    # max(cntA) + max(cntB) (jagged layout bills each round individually)
    A0r_ = np.zeros(R, dtype=np.int64)
    B0r_ = np.zeros(R, dtype=np.int64)
    Mr_ = np.zeros(R, dtype=np.int64)
    np.maximum.at(A0r_, rnd_of_node_row, nA0)
    np.maximum.at(B0r_, rnd_of_node_row, nB0)
    np.maximum.at(Mr_, rnd_of_node_row, degr)
    costr = np.maximum(Mr_, A0r_ + B0r_)
    TAr = np.clip((costr + 1) // 2, A0r_, costr - B0r_)
    TBr = costr - TAr
    r_of_row = rnd_of_node_row
    lo = np.maximum(nA0, degr - TBr[r_of_row])
    hi = np.minimum(TAr[r_of_row], nA0 + nf)
    cntA = np.clip((degr + 1) // 2, lo, hi)

    o = np.argsort(dr[flex], kind="stable")
    flex_idx = np.nonzero(flex)[0][o]
    grp = dr[flex_idx]
    uniq, first = np.unique(grp, return_index=True)
    fr = np.arange(len(grp)) - first[np.searchsorted(uniq, grp)]
    bank = np.ones(E, dtype=np.int8)
    bank[forcedA] = 0
    bank[flex_idx] = (fr >= (cntA[grp] - nA0[grp])).astype(np.int8)
    cntB = degr - cntA

    DAr = np.zeros(R, dtype=np.int64)
    DBr = np.zeros(R, dtype=np.int64)
    np.maximum.at(DAr, rnd_of_node_row, cntA)
    np.maximum.at(DBr, rnd_of_node_row, cntB)
    # rect-A slot d=0 must exist in every round: the group's first matmul
    # (bank A, d=0) covers all po columns with start=True
    DAr = np.maximum(DAr, 1)

    # slot position within (dst, bank); negative gather indices first so the
    # final slot of each (dst, bank) sequence is non-negative where possible
    idxval = np.where(bank == 0, sr - baseA, sr - baseB)
    nonneg = (idxval >= 0).astype(np.int8)
    o2 = np.lexsort((nonneg, bank, dr))
    grp2 = dr[o2] * 2 + bank[o2]
    uniq2, first2 = np.unique(grp2, return_index=True)
    dpos = np.arange(E) - first2[np.searchsorted(uniq2, grp2)]
    d_of_edge = np.empty(E, dtype=np.int64)
    d_of_edge[o2] = dpos

    cnt_nonneg_A = np.bincount(dr[(bank == 0) & (idxval >= 0)], minlength=NTOT)
    cnt_nonneg_B = np.bincount(dr[(bank == 1) & (idxval >= 0)], minlength=NTOT)
    p_of_row = np.empty(NTOT, dtype=np.int64)
    p_of_row[row_of_listpos] = p
    is_last_p = p_of_row == 127

    def build_plan(Dr, cnt, cnt_nonneg):
        # jagged layout per (group, bank): rect core (d < dmin over the
        # group's rounds) + per-d tail runs of rounds still alive. Returns
        # (plans, offsets); bumps Dr where the call's final slot would be a
        # full all-negative (dst,bank) sequence (HW drops trailing negatives).
        while True:
            plans = []
            off = [0]
            redo = False
            for g, (g0, gl) in enumerate(groups):
                D = Dr[g0:g0 + gl]
                dmin = int(D.min())
                S = dmin * gl
                runs = []
                last_rl = gl - 1
                for d in range(dmin, int(D.max())):
                    rl = 0
                    while rl < gl:
                        if D[rl] > d:
                            rl0 = rl
                            while rl < gl and D[rl] > d:
                                rl += 1
                            runs.append((d, rl0, rl - rl0, S))
                            S += rl - rl0
                            last_rl = rl - 1
                        else:
                            rl += 1
                plans.append(dict(dmin=dmin, S=S, runs=runs))
                off.append(off[-1] + S)
                # guard: dst at (round of final block, p=127) must not have a
                # full all-negative slot sequence
                rr = g0 + last_rl
                sel = is_last_p & (rnd_of_node_row == rr)
                if np.any(sel & (cnt == Dr[rr]) & (cnt > 0) & (cnt_nonneg == 0)):
                    Dr[rr] += 1
                    redo = True
                    break
            if not redo:
                return plans, off

    planA, offA = build_plan(DAr, cntA, cnt_nonneg_A)
    planB, offB = build_plan(DBr, cntB, cnt_nonneg_B)
    SA, SB = int(offA[-1]), int(offB[-1])

    def blk_lut(Dr, plans, off):
        maxD = max(int(Dr.max()), 1)
        lut = np.full((R, maxD), -1, dtype=np.int64)
        for g, (g0, gl) in enumerate(groups):
            pl = plans[g]
            dmin = pl["dmin"]
            for rl in range(gl):
                for d in range(dmin):
                    lut[g0 + rl, d] = off[g] + d * gl + rl
            for (d, rl0, n, blk0) in pl["runs"]:
                for j in range(n):
                    lut[g0 + rl0 + j, d] = off[g] + blk0 + j
        return lut

    lutA = blk_lut(DAr, planA, offA)
    lutB = blk_lut(DBr, planB, offB)

    idxA = np.zeros((C, SA * 128), dtype=np.int32)
    idxB = np.zeros((C, SB * 128), dtype=np.int32)
    e_half = dr // (C * RH * 128)
    e_rem = dr % (C * RH * 128)
    e_core = e_rem // (RH * 128)
    e_rnd = e_half * RH + (e_rem % (RH * 128)) // 128
    e_p = dr % 128
    isA = bank == 0
    blkA = lutA[e_rnd[isA], d_of_edge[isA]]
    assert (blkA >= 0).all()
    idxA[e_core[isA], blkA * 128 + e_p[isA]] = sr[isA] - baseA
    isB = ~isA
    blkB = lutB[e_rnd[isB], d_of_edge[isB]]
    assert (blkB >= 0).all()
    idxB[e_core[isB], blkB * 128 + e_p[isB]] = sr[isB] - baseB
    assert idxA.min() >= -32768 and idxA.max() <= 32766
    assert idxB.min() >= -32768 and idxB.max() <= 32766
    # final slot of every call must be non-negative (trailing negatives drop)
    for g in range(NG):
        if offA[g + 1] > offA[g]:
            assert (idxA[:, offA[g + 1] * 128 - 1] >= 0).all()
        if offB[g + 1] > offB[g]:
            assert (idxB[:, offB[g + 1] * 128 - 1] >= 0).all()

    def wrap(a):  # [C, S*128] -> [C, 128, S*8] int16 (16-wrap, replicated x8)
        Cn, tot = a.shape
        if tot == 0:
            return np.zeros((Cn, 128, 0), dtype=np.int16)
        w = a.reshape(Cn, tot // 16, 16).transpose(0, 2, 1)
        return np.ascontiguousarray(np.tile(w, (1, 8, 1))).astype(np.int16)

    # shard-local position (round-major) differs from the half-major table row
    shardpos = 128 * rnd + p
    xT = np.zeros((C, x.shape[1], NPC), dtype=np.float32)
    xf = np.asarray(x, dtype=np.float32)
    for k in range(C):
        sel = (core == k) & real
        xT[k][:, shardpos[sel]] = xf[node_at_listpos[sel]].T

    # host-side gather index: node -> (core, shardpos) in concatenated output
    outpos_of_listpos = core * NPC + shardpos
    outpos_of_node = np.full(N, -1, dtype=np.int64)
    outpos_of_node[node_at_listpos[real]] = outpos_of_listpos[real]

    return dict(
        idxA=wrap(idxA), idxB=wrap(idxB), xT=xT,
        planA=planA, planB=planB,
        offA=[int(v) for v in offA], offB=[int(v) for v in offB],
        groups=groups, SA=SA, SB=SB, RH=RH,
        row_of_node=outpos_of_node, table_row_of_node=row_of_node,
    )


def _plan_key(plans):
    return tuple((p["dmin"], p["S"], tuple(p["runs"])) for p in plans)


def _build(cfg, planA, planB, offA, offB, groups, SA, SB, RH):
    import sys
    if "/opt/trn_rl_repo" not in sys.path:
        sys.path.insert(0, "/opt/trn_rl_repo")
    import concourse.mybir as mybir
    import concourse.tile as tile
    from concourse import bacc
    from concourse.masks import make_identity

    f32 = mybir.dt.float32
    bf16 = mybir.dt.bfloat16
    R, GL = cfg["R"], cfg["GL"]
    F, HD = cfg["F"], cfg["H"]
    NPC = R * 128
    NTOT = 8 * NPC
    baseA, baseB = cfg["baseA"], cfg["baseB"]
    AF = HD + 2  # h | asrc | adst (f32 table-build layout)
    NG = len(groups)
    NPCa = RH * 128          # shard half sizes (round-major)
    NPCb = NPC - NPCa

    nc = bacc.Bacc("TRN2", target_bir_lowering=False, debug=False, num_devices=8)
    xT_t = nc.dram_tensor("xT", [F, NPC], f32, kind="ExternalInput")
    iA_t = nc.dram_tensor("idxA", [128, SA * 8], mybir.dt.int16, kind="ExternalInput")
    iB_t = nc.dram_tensor("idxB", [128, SB * 8], mybir.dt.int16, kind="ExternalInput")
    W1_t = nc.dram_tensor("W1", [F, HD], f32, kind="ExternalInput")
    W2_t = nc.dram_tensor("W2", [HD, HD], f32, kind="ExternalInput")
    av_t = nc.dram_tensor("avec", [4, HD], f32, kind="ExternalInput")
    bv_t = nc.dram_tensor("bvec", [2, HD], f32, kind="ExternalInput")
    out_t = nc.dram_tensor("out", [NPC, HD], f32, kind="ExternalOutput")

    # shard halves are separate tensors so the AllGather of half a never
    # false-depends on phase-A writes of half b
    shard1a = nc.dram_tensor("shard1a", [NPCa, 128], bf16, kind="Internal")
    shard2a = nc.dram_tensor("shard2a", [NPCa, 128], bf16, kind="Internal")
    if NPCb:
        shard1b = nc.dram_tensor("shard1b", [NPCb, 128], bf16, kind="Internal")
        shard2b = nc.dram_tensor("shard2b", [NPCb, 128], bf16, kind="Internal")
    else:
        shard1b = shard2b = None
    table1 = nc.dram_tensor("table1", [NTOT, 128], bf16, kind="Internal",
                            addr_space="Shared")
    table2 = nc.dram_tensor("table2", [NTOT, 128], bf16, kind="Internal",
                            addr_space="Shared")
    shards = {(1, 0): shard1a, (1, 1): shard1b, (2, 0): shard2a, (2, 1): shard2b}

    RG = [[0, 1, 2, 3, 4, 5, 6, 7]]

    with tile.TileContext(nc) as tc:
        with tc.tile_pool(name="const", bufs=1) as cp, \
             tc.tile_pool(name="spool", bufs=3) as sp, \
             tc.tile_pool(name="gpool", bufs=3) as gp, \
             tc.tile_pool(name="mpool", bufs=2) as mp, \
             tc.tile_pool(name="hpool", bufs=NG) as hp, \
             tc.tile_pool(name="psA", bufs=2, space="PSUM") as psA, \
             tc.tile_pool(name="psT", bufs=2, space="PSUM") as psT, \
             tc.tile_pool(name="psO", bufs=2, space="PSUM") as psO:

            ident = cp.tile([128, 128], f32)
            make_identity(nc, ident[:])
            identb = cp.tile([128, 128], bf16)
            make_identity(nc, identb[:])

            # weight prep: aug[l] = [W | W@a_src | W@a_dst]  ([K, AF])
            augs = []
            for l, (Wt, K) in enumerate(((W1_t, F), (W2_t, HD))):
                Wsb = cp.tile([K, HD], f32, tag=f"w{l}")
                nc.sync.dma_start(out=Wsb[:], in_=Wt.ap()[:, :])
                Wt_ps = psT.tile([HD, K], f32, tag="pst")
                nc.tensor.transpose(out=Wt_ps[:], in_=Wsb[:], identity=ident[:K, :K])
                Wtr = cp.tile([HD, K], f32, tag=f"wt{l}")
                nc.vector.tensor_copy(out=Wtr[:], in_=Wt_ps[:])
                aug = cp.tile([K, AF], f32, tag=f"aug{l}")
                nc.vector.tensor_copy(out=aug[:, 0:HD], in_=Wsb[:])
                for s in range(2):
                    acol = cp.tile([HD, 1], f32, tag=f"ac{l}{s}")
                    nc.sync.dma_start(
                        out=acol[:],
                        in_=av_t.ap()[2 * l + s:2 * l + s + 1, :].rearrange("a b -> b a"))
                    wa_ps = psT.tile([K, 1], f32, tag="pst")
                    nc.tensor.matmul(out=wa_ps[:], lhsT=Wtr[:], rhs=acol[:],
                                     start=True, stop=True)
                    nc.vector.tensor_copy(out=aug[:, HD + s:HD + s + 1], in_=wa_ps[:])
                augs.append(aug)

            # bias, replicated GL times: bbg[l] = [128, GL, HD]
            bbg = []
            for l in range(2):
                t = cp.tile([128, HD], f32, tag=f"b{l}")
                nc.sync.dma_start(out=t[:1, :], in_=bv_t.ap()[l:l + 1, :])
                nc.gpsimd.partition_broadcast(t[:], t[:1, :])
                tg = cp.tile([128, GL, HD], f32, tag=f"bg{l}")
                for rl in range(GL):
                    nc.vector.tensor_copy(out=tg[:, rl, :], in_=t[:])
                bbg.append(tg)

            # dummy row: h = 0, asrc = -1e30 (f32 at bf16 cols 64:66)
            dumrow = cp.tile([1, 128], bf16)
            nc.vector.memset(dumrow[:], 0.0)
            nc.vector.memset(dumrow[:, 64:66].bitcast(f32), -1e30)

            iA_sb = cp.tile([128, SA * 8], mybir.dt.int16)
            nc.sync.dma_start(out=iA_sb[:], in_=iA_t.ap()[:, :])
            iB_sb = cp.tile([128, SB * 8], mybir.dt.int16)
            nc.sync.dma_start(out=iB_sb[:], in_=iB_t.ap()[:, :])

            adst_own1 = cp.tile([128, R], f32, tag="adst1")
            adst_own2 = cp.tile([128, R], f32, tag="adst2")
            adst_own = [adst_own1, adst_own2]

            def table_chunk_write(i, hs, lnum, layer):
                # hs: SBUF [AF, 128] f-major -> bf16 node-major rows
                htp = psT.tile([128, AF], f32, tag="pst")
                nc.tensor.transpose(out=htp[:], in_=hs[:], identity=ident[:AF, :AF])
                chunk = sp.tile([128, 128], bf16, tag="chunk")
                nc.vector.tensor_copy(out=chunk[:, 0:HD], in_=htp[:, 0:HD])
                nc.vector.tensor_copy(out=chunk[:, 64:66].bitcast(f32),
                                      in_=htp[:, HD:HD + 1])
                nc.vector.tensor_copy(out=chunk[:, 66:68].bitcast(f32),
                                      in_=htp[:, HD + 1:HD + 2])
                nc.vector.tensor_copy(out=adst_own[layer][:, i:i + 1],
                                      in_=htp[:, HD + 1:HD + 2])
                hf, ro = (0, i) if i < RH else (1, i - RH)
                nc.sync.dma_start(
                    out=shards[(lnum, hf)].ap()[128 * ro:128 * (ro + 1), :],
                    in_=chunk[:])

            def allgather_half(lnum, table, hf):
                n_in = NPCa if hf == 0 else NPCb
                if n_in == 0:
                    return
                shard = shards[(lnum, hf)]
                o0 = 0 if hf == 0 else 8 * NPCa
                nc.gpsimd.collective_compute(
                    "AllGather", mybir.AluOpType.bypass, RG,
                    ins=[shard.ap()[:, :]],
                    outs=[table.ap()[o0:o0 + 8 * n_in, :]])
                for base in (baseA, baseB):
                    if o0 <= base < o0 + 8 * n_in:
                        nc.gpsimd.dma_start(out=table.ap()[base:base + 1, :],
                                            in_=dumrow[:])

            def phase_A1():
                for t in range(R):
                    rhs = sp.tile([F, 128], f32, tag="parhs")
                    nc.sync.dma_start(out=rhs[:], in_=xT_t.ap()[:, 128 * t:128 * (t + 1)])
                    hp_ = psA.tile([AF, 128], f32, tag="paps")
                    nc.tensor.matmul(out=hp_[:], lhsT=augs[0][:], rhs=rhs[:],
                                     start=True, stop=True)
                    hs = sp.tile([AF, 128], f32, tag="pahs")
                    nc.scalar.copy(out=hs[:], in_=hp_[:])
                    table_chunk_write(t, hs, 1, 0)
                    if t == RH - 1:
                        allgather_half(1, table1, 0)
                allgather_half(1, table1, 1)

            def phase_B_group(layer, g, table, adst):
                final = layer == 1
                g0, gl = groups[g]
                plA, plB = planA[g], planB[g]
                BA, BB = plA["S"], plB["S"]
                GA = gp.tile([128, max(BA, 1), 128], bf16, tag="G")
                if BA:
                    nc.gpsimd.dma_gather(
                        out_ap=GA[:, 0:BA, :], in_ap=table.ap()[baseA:, :],
                        idxs_ap=iA_sb[:, offA[g] * 8:offA[g + 1] * 8],
                        num_idxs=128 * BA, num_idxs_reg=128 * BA,
                        elem_size=128, single_packet=False)
                GB = gp.tile([128, max(BB, 1), 128], bf16, tag="G")
                if BB:
                    nc.gpsimd.dma_gather(
                        out_ap=GB[:, 0:BB, :], in_ap=table.ap()[baseB:, :],
                        idxs_ap=iB_sb[:, offB[g] * 8:offB[g + 1] * 8],
                        num_idxs=128 * BB, num_idxs_reg=128 * BB,
                        elem_size=128, single_packet=False)

                po = psO.tile([128, gl, HD + 1], f32, tag="po")
                nmm = sum(pl["dmin"] + len(pl["runs"])
                          for pl, B in ((plA, BA), (plB, BB)) if B)
                mm_i = 0
                for (G, pl, B, btag) in ((GA, plA, BA, "a"), (GB, plB, BB, "b")):
                    if B == 0:
                        continue
                    dmin, runs = pl["dmin"], pl["runs"]
                    # scores on the asrc (f32) subfield; adst replicated per
                    # round across the interleaved rect blocks + tail runs
                    arep = mp.tile([128, B, 1], f32, tag="arep" + btag)
                    if dmin:
                        ar4 = arep[:, 0:dmin * gl, :].rearrange(
                            "p (d r) o -> p d r o", d=dmin)
                        for rl in range(gl):
                            nc.vector.tensor_copy(
                                out=ar4[:, 0:dmin, rl, 0],
                                in_=adst[:, g0 + rl:g0 + rl + 1].to_broadcast(
                                    [128, dmin]))
                    for (d, rl0, n, blk0) in runs:
                        nc.vector.tensor_copy(
                            out=arep[:, blk0:blk0 + n, 0],
                            in_=adst[:, g0 + rl0:g0 + rl0 + n])
                    zt = mp.tile([128, B, 1], f32, tag="zt" + btag)
                    nc.vector.tensor_tensor(
                        out=zt[:, 0:B, :], in0=G[:, 0:B, 64:66].bitcast(f32),
                        in1=arep[:, 0:B, :], op=mybir.AluOpType.add)
                    z2 = mp.tile([128, B, 1], f32, tag="z2" + btag)
                    nc.vector.tensor_scalar(
                        out=z2[:, 0:B, :], in0=zt[:, 0:B, :],
                        scalar1=cfg["slope"], scalar2=None,
                        op0=mybir.AluOpType.mult)
                    lt = mp.tile([128, B, 1], f32, tag="lt" + btag)
                    nc.vector.tensor_tensor(
                        out=lt[:, 0:B, :], in0=zt[:, 0:B, :], in1=z2[:, 0:B, :],
                        op=mybir.AluOpType.max)
                    # t (bf16) overwrites the asrc-lo slot -> col 64
                    nc.scalar.activation(
                        out=G[:, 0:B, 64:65], in_=lt[:, 0:B, :],
                        func=mybir.ActivationFunctionType.Exp)
                    # weighted messages in place: h *= t
                    nc.vector.tensor_tensor(
                        out=G[:, 0:B, 0:HD], in0=G[:, 0:B, 0:HD],
                        in1=G[:, 0:B, 64:65].to_broadcast([128, B, HD]),
                        op=mybir.AluOpType.mult)
                    if dmin:
                        G4 = G[:, 0:dmin * gl, 0:HD + 1].rearrange(
                            "p (d r) f -> p d r f", d=dmin)
                        for d in range(dmin):
                            nc.tensor.matmul(
                                out=po[:, 0:gl, :], lhsT=identb[:],
                                rhs=G4[:, d, :, :],
                                start=mm_i == 0, stop=mm_i == nmm - 1)
                            mm_i += 1
                    for (d, rl0, n, blk0) in runs:
                        nc.tensor.matmul(
                            out=po[:, rl0:rl0 + n, :], lhsT=identb[:],
                            rhs=G[:, blk0:blk0 + n, 0:HD + 1],
                            start=mm_i == 0, stop=mm_i == nmm - 1,
                            skip_group_check=True)
                        mm_i += 1

                den = mp.tile([128, gl, 1], f32, tag="den")
                nc.vector.tensor_scalar_max(out=den[:, 0:gl, :],
                                            in0=po[:, 0:gl, HD:HD + 1],
                                            scalar1=1e-16)
                rd = mp.tile([128, gl, 1], f32, tag="rd")
                nc.vector.reciprocal(out=rd[:, 0:gl, :], in_=den[:, 0:gl, :])
                h = (mp if final else hp).tile([128, gl, HD], f32,
                                               tag="hfin" + str(layer))
                nc.vector.tensor_tensor(
                    out=h[:, 0:gl, :], in0=po[:, 0:gl, 0:HD],
                    in1=rd[:, 0:gl, :].to_broadcast([128, gl, HD]),
                    op=mybir.AluOpType.mult)
                nc.vector.tensor_tensor(out=h[:, 0:gl, :], in0=h[:, 0:gl, :],
                                        in1=bbg[layer][:, 0:gl, :],
                                        op=mybir.AluOpType.add)
                if final:
                    nc.sync.dma_start(
                        out=out_t.ap()[128 * g0:128 * (g0 + gl), :].rearrange(
                            "(r p) f -> p r f", r=gl),
                        in_=h[:, 0:gl, :])
                else:
                    nc.scalar.activation(out=h[:, 0:gl, :], in_=h[:, 0:gl, :],
                                         func=mybir.ActivationFunctionType.Relu)
                    hkeep.append(h)

            def phase_A2_group(g):
                g0, gl = groups[g]
                h = hkeep[g]
                for rl in range(gl):
                    t = g0 + rl
                    htr = psT.tile([HD, 128], f32, tag="pst")
                    nc.tensor.transpose(out=htr[:], in_=h[:, rl, :],
                                        identity=ident[:])
                    ht = sp.tile([HD, 128], f32, tag="hTs")
                    nc.scalar.copy(out=ht[:], in_=htr[:])
                    hp2 = psA.tile([AF, 128], f32, tag="paps")
                    nc.tensor.matmul(out=hp2[:], lhsT=augs[1][:], rhs=ht[:],
                                     start=True, stop=True)
                    hs2 = sp.tile([AF, 128], f32, tag="pahs")
                    nc.scalar.copy(out=hs2[:], in_=hp2[:])
                    table_chunk_write(t, hs2, 2, 1)

            hkeep = []
            phase_A1()
            ag2a_done = False
            for g in range(NG):
                phase_B_group(0, g, table1, adst_own[0])
                phase_A2_group(g)
                g0, gl = groups[g]
                if not ag2a_done and g0 + gl >= RH:
                    allgather_half(2, table2, 0)
                    ag2a_done = True
            allgather_half(2, table2, 1)
            for g in range(NG):
                phase_B_group(1, g, table2, adst_own[1])

    nc.compile()
    return nc


def _make_cfg(N, F, H):
    if N >= 32768:
        return dict(N=N, R=98, GL=7, baseA=32768, baseB=67585, span=32766,
                    F=F, H=H, slope=0.2)
    NTOT = max(2048, ((N + 128 + 1023) // 1024) * 1024)
    R = NTOT // 1024
    return dict(N=N, R=R, GL=min(7, R), baseA=NTOT // 4, baseB=(3 * NTOT) // 4,
                span=min(32766, (5 * NTOT) // 8), F=F, H=H, slope=0.2)


def _make_in_maps(inputs, prep):
    avec = np.stack([np.asarray(inputs["a1_src"]), np.asarray(inputs["a1_dst"]),
                     np.asarray(inputs["a2_src"]), np.asarray(inputs["a2_dst"])]
                    ).astype(np.float32)
    bvec = np.stack([np.asarray(inputs["b1"]), np.asarray(inputs["b2"])]
                    ).astype(np.float32)
    in_maps = []
    for k in range(8):
        in_maps.append({
            "xT": prep["xT"][k], "idxA": prep["idxA"][k], "idxB": prep["idxB"][k],
            "W1": np.asarray(inputs["W1"], dtype=np.float32),
            "W2": np.asarray(inputs["W2"], dtype=np.float32),
            "avec": avec, "bvec": bvec,
        })
    return in_maps


def kernel(x, edge_index, W1, a1_src, a1_dst, b1, W2, a2_src, a2_dst, b2):
    import sys
    if "/opt/trn_rl_repo" not in sys.path:
        sys.path.insert(0, "/opt/trn_rl_repo")
    from concourse import bass_utils

    x = np.asarray(x)
    cfg = _make_cfg(x.shape[0], x.shape[1], np.asarray(W1).shape[1])
    prep = _host_prep(x, edge_index, cfg)
    key = (cfg["N"], cfg["R"], prep["SA"], prep["SB"],
           _plan_key(prep["planA"]), _plan_key(prep["planB"]))
    if key not in _CACHE:
        _CACHE[key] = _build(cfg, prep["planA"], prep["planB"],
                             prep["offA"], prep["offB"], prep["groups"],
                             prep["SA"], prep["SB"], prep["RH"])
    nc = _CACHE[key]

    in_maps = _make_in_maps(
        dict(a1_src=a1_src, a1_dst=a1_dst, a2_src=a2_src, a2_dst=a2_dst,
             b1=b1, b2=b2, W1=W1, W2=W2), prep)
    res = bass_utils.run_bass_kernel_spmd(nc, in_maps, core_ids=list(range(8)))
    shards = np.concatenate([res.results[k]["out"] for k in range(8)], axis=0)
    return shards[prep["row_of_node"]].astype(np.float32)


# revision 27
# speedup vs baseline: 58.7064x; 58.1492x over previous
"""Two-layer GAT on 8 Trainium2 NeuronCores.

Strategy (dst-sharded, node-major bf16 table):
 - Nodes are degree-sorted into 128-node blocks; blocks are dealt round-robin
   to the 8 cores so every core runs an identical static schedule. Rounds are
   grouped GL=7 at a time; one PSUM bank accumulates all 7 rounds (7*65=455
   cols <= 512).
 - Per layer, each core computes its shard of a node table
   [row: h(64 bf16) | asrc(f32) | adst(f32) | pad -> 128 bf16 = 256B], an
   AllGather replicates the full table, then each (group, bank) issues ONE
   dma_gather whose index order interleaves the group's rounds
   (block = slot_d * gl + round_local), so slot-d of all 7 rounds is one
   matmul rhs [128, gl, 65] accumulating into po[128, gl, 65].
 - t_e = exp(leaky_relu(asrc_src + adst_dst)) is computed on the gathered
   chunk in a handful of wide vector ops; t overwrites the asrc slot (bf16)
   so rhs cols 0:65 are [t*h | t] after one broadcast multiply.
 - Softmax max-subtraction is algebraically a no-op here (scores are O(10)).
   Padded slots gather a dummy table row with asrc = -1e30 so t == 0.
 - Two int16-index banks (A at row baseA, B at baseB) cover the >64K rows.
"""
import numpy as np

_CACHE = {}


def _host_prep(x, edge_index, cfg):
    N, C, R, GL = cfg["N"], 8, cfg["R"], cfg["GL"]
    NPC = R * 128            # rows per core shard
    NTOT = C * NPC
    # rounds in table half 0 (AllGather split point); RH == R disables split
    # (shared-output AllGather makes the split a wash on HW, and fewer/bigger
    # collectives are strictly better in the cost model)
    RH = (R + 1) // 2 if cfg.get("agsplit", False) else R
    baseA, baseB, span = cfg["baseA"], cfg["baseB"], cfg["span"]
    A_hi = min(NTOT - 1, baseA + span)
    B_lo = max(0, baseB - span)
    assert A_hi >= B_lo - 1

    groups = [(i * GL, min(GL, R - i * GL)) for i in range((R + GL - 1) // GL)]
    NG = len(groups)

    src = np.asarray(edge_index[0], dtype=np.int64)
    dst = np.asarray(edge_index[1], dtype=np.int64)
    E = src.shape[0]

    deg = np.bincount(dst, minlength=N)
    odeg = np.bincount(src, minlength=N)
    order = np.argsort(-deg, kind="stable")
    all_nodes = np.concatenate([order, np.full(NTOT - N, -1, dtype=np.int64)])

    m = np.arange(NTOT)
    b = m // 128
    p = m % 128
    rnd = b // C
    core = b % C
    # half-major row layout so each AllGather half is a contiguous table range
    half = rnd // RH
    rw = rnd % RH
    row_of_listpos = (half * (C * RH * 128) + core * (RH * 128) + 128 * rw + p)

    # within each round, put the highest OUT-degree nodes on rows inside the
    # flex window [B_lo, A_hi] — their out-edges become bank-flexible, which
    # shrinks the forced-bank count tails that drive slot padding
    for r in range(R):
        sel = np.flatnonzero(rnd == r)
        rows = row_of_listpos[sel]
        flex = (rows >= B_lo) & (rows <= A_hi)
        nd = all_nodes[sel]
        od = np.where(nd >= 0, odeg[np.clip(nd, 0, None)], -1)
        pos_order = np.argsort(~flex, kind="stable")   # flex positions first
        nd_order = np.argsort(-od, kind="stable")      # high out-degree first
        newnd = np.empty_like(nd)
        newnd[pos_order] = nd[nd_order]
        all_nodes[sel] = newnd

    # bank holes + dummy rows must hold pad nodes
    special_rows = {baseA - 1, baseA, baseB - 1, baseB}
    row_to_listpos = np.empty(NTOT, dtype=np.int64)
    row_to_listpos[row_of_listpos] = m
    pad_positions = [i for i in range(NTOT - 1, -1, -1) if all_nodes[i] < 0]
    pi = 0
    for r in special_rows:
        lp = row_to_listpos[r]
        if all_nodes[lp] >= 0:
            while pi < len(pad_positions):
                q = pad_positions[pi]; pi += 1
                if row_of_listpos[q] not in special_rows and all_nodes[q] < 0:
                    all_nodes[lp], all_nodes[q] = all_nodes[q], all_nodes[lp]
                    break

    node_at_listpos = all_nodes
    row_of_node = np.full(N, -1, dtype=np.int64)
    real = node_at_listpos >= 0
    row_of_node[node_at_listpos[real]] = row_of_listpos[real]

    sr = row_of_node[src]
    dr = row_of_node[dst]

    rnd_of_node_row = np.empty(NTOT, dtype=np.int64)
    rnd_of_node_row[row_of_listpos] = rnd

    # bank per edge (0=A, 1=B); per-group thresholds TA/TB minimize the
    # rectangular slot count max(cntA) + max(cntB) over the group
    canA = sr <= A_hi
    canB = sr >= B_lo
    forcedA = canA & ~canB
    forcedB = ~canA & canB
    flex = canA & canB
    nA0 = np.bincount(dr[forcedA], minlength=NTOT)
    nB0 = np.bincount(dr[forcedB], minlength=NTOT)
    nf = np.bincount(dr[flex], minlength=NTOT)
    degr = np.bincount(dr, minlength=NTOT)

    # per-round thresholds TA/TB minimize the per-round slot count
    # max(cntA) + max(cntB) (jagged layout bills each round individually)
    A0r_ = np.zeros(R, dtype=np.int64)
    B0r_ = np.zeros(R, dtype=np.int64)
    Mr_ = np.zeros(R, dtype=np.int64)
    np.maximum.at(A0r_, rnd_of_node_row, nA0)
    np.maximum.at(B0r_, rnd_of_node_row, nB0)
    np.maximum.at(Mr_, rnd_of_node_row, degr)
    costr = np.maximum(Mr_, A0r_ + B0r_)
    TAr = np.clip((costr + 1) // 2, A0r_, costr - B0r_)
    TBr = costr - TAr
    r_of_row = rnd_of_node_row
    lo = np.maximum(nA0, degr - TBr[r_of_row])
    hi = np.minimum(TAr[r_of_row], nA0 + nf)
    cntA = np.clip((degr + 1) // 2, lo, hi)

    o = np.argsort(dr[flex], kind="stable")
    flex_idx = np.nonzero(flex)[0][o]
    grp = dr[flex_idx]
    uniq, first = np.unique(grp, return_index=True)
    fr = np.arange(len(grp)) - first[np.searchsorted(uniq, grp)]
    bank = np.ones(E, dtype=np.int8)
    bank[forcedA] = 0
    bank[flex_idx] = (fr >= (cntA[grp] - nA0[grp])).astype(np.int8)
    cntB = degr - cntA

    DAr = np.zeros(R, dtype=np.int64)
    DBr = np.zeros(R, dtype=np.int64)
    np.maximum.at(DAr, rnd_of_node_row, cntA)
    np.maximum.at(DBr, rnd_of_node_row, cntB)
    # rect-A slot d=0 must exist in every round: the group's first matmul
    # (bank A, d=0) covers all po columns with start=True
    DAr = np.maximum(DAr, 1)

    # slot position within (dst, bank); negative gather indices first so the
    # final slot of each (dst, bank) sequence is non-negative where possible
    idxval = np.where(bank == 0, sr - baseA, sr - baseB)
    nonneg = (idxval >= 0).astype(np.int8)
    o2 = np.lexsort((nonneg, bank, dr))
    grp2 = dr[o2] * 2 + bank[o2]
    uniq2, first2 = np.unique(grp2, return_index=True)
    dpos = np.arange(E) - first2[np.searchsorted(uniq2, grp2)]
    d_of_edge = np.empty(E, dtype=np.int64)
    d_of_edge[o2] = dpos

    cnt_nonneg_A = np.bincount(dr[(bank == 0) & (idxval >= 0)], minlength=NTOT)
    cnt_nonneg_B = np.bincount(dr[(bank == 1) & (idxval >= 0)], minlength=NTOT)
    p_of_row = np.empty(NTOT, dtype=np.int64)
    p_of_row[row_of_listpos] = p
    is_last_p = p_of_row == 127

    def build_plan(Dr, cnt, cnt_nonneg):
        # jagged layout per (group, bank): rect core (d < dmin over the
        # group's rounds) + per-d tail runs of rounds still alive. Returns
        # (plans, offsets); bumps Dr where the call's final slot would be a
        # full all-negative (dst,bank) sequence (HW drops trailing negatives).
        while True:
            plans = []
            off = [0]
            redo = False
            for g, (g0, gl) in enumerate(groups):
                D = Dr[g0:g0 + gl]
                dmin = int(D.min())
                S = dmin * gl
                runs = []
                last_rl = gl - 1
                for d in range(dmin, int(D.max())):
                    rl = 0
                    while rl < gl:
                        if D[rl] > d:
                            rl0 = rl
                            while rl < gl and D[rl] > d:
                                rl += 1
                            runs.append((d, rl0, rl - rl0, S))
                            S += rl - rl0
                            last_rl = rl - 1
                        else:
                            rl += 1
                plans.append(dict(dmin=dmin, S=S, runs=runs))
                off.append(off[-1] + S)
                # guard: dst at (round of final block, p=127) must not have a
                # full all-negative slot sequence
                rr = g0 + last_rl
                sel = is_last_p & (rnd_of_node_row == rr)
                if np.any(sel & (cnt == Dr[rr]) & (cnt > 0) & (cnt_nonneg == 0)):
                    Dr[rr] += 1
                    redo = True
                    break
            if not redo:
                return plans, off

    planA, offA = build_plan(DAr, cntA, cnt_nonneg_A)
    planB, offB = build_plan(DBr, cntB, cnt_nonneg_B)
    SA, SB = int(offA[-1]), int(offB[-1])

    def blk_lut(Dr, plans, off):
        maxD = max(int(Dr.max()), 1)
        lut = np.full((R, maxD), -1, dtype=np.int64)
        for g, (g0, gl) in enumerate(groups):
            pl = plans[g]
            dmin = pl["dmin"]
            for rl in range(gl):
                for d in range(dmin):
                    lut[g0 + rl, d] = off[g] + d * gl + rl
            for (d, rl0, n, blk0) in pl["runs"]:
                for j in range(n):
                    lut[g0 + rl0 + j, d] = off[g] + blk0 + j
        return lut

    lutA = blk_lut(DAr, planA, offA)
    lutB = blk_lut(DBr, planB, offB)

    idxA = np.zeros((C, SA * 128), dtype=np.int32)
    idxB = np.zeros((C, SB * 128), dtype=np.int32)
    e_half = dr // (C * RH * 128)
    e_rem = dr % (C * RH * 128)
    e_core = e_rem // (RH * 128)
    e_rnd = e_half * RH + (e_rem % (RH * 128)) // 128
    e_p = dr % 128
    isA = bank == 0
    blkA = lutA[e_rnd[isA], d_of_edge[isA]]
    assert (blkA >= 0).all()
    idxA[e_core[isA], blkA * 128 + e_p[isA]] = sr[isA] - baseA
    isB = ~isA
    blkB = lutB[e_rnd[isB], d_of_edge[isB]]
    assert (blkB >= 0).all()
    idxB[e_core[isB], blkB * 128 + e_p[isB]] = sr[isB] - baseB
    assert idxA.min() >= -32768 and idxA.max() <= 32766
    assert idxB.min() >= -32768 and idxB.max() <= 32766
    # final slot of every call must be non-negative (trailing negatives drop)
    for g in range(NG):
        if offA[g + 1] > offA[g]:
            assert (idxA[:, offA[g + 1] * 128 - 1] >= 0).all()
        if offB[g + 1] > offB[g]:
            assert (idxB[:, offB[g + 1] * 128 - 1] >= 0).all()

    def wrap(a):  # [C, S*128] -> [C, 128, S*8] int16 (16-wrap, replicated x8)
        Cn, tot = a.shape
        if tot == 0:
            return np.zeros((Cn, 128, 0), dtype=np.int16)
        w = a.reshape(Cn, tot // 16, 16).transpose(0, 2, 1)
        return np.ascontiguousarray(np.tile(w, (1, 8, 1))).astype(np.int16)

    # shard-local position (round-major) differs from the half-major table row
    shardpos = 128 * rnd + p
    xT = np.zeros((C, x.shape[1], NPC), dtype=np.float32)
    xf = np.asarray(x, dtype=np.float32)
    for k in range(C):
        sel = (core == k) & real
        xT[k][:, shardpos[sel]] = xf[node_at_listpos[sel]].T

    # host-side gather index: node -> (core, shardpos) in concatenated output
    outpos_of_listpos = core * NPC + shardpos
    outpos_of_node = np.full(N, -1, dtype=np.int64)
    outpos_of_node[node_at_listpos[real]] = outpos_of_listpos[real]

    return dict(
        idxA=wrap(idxA), idxB=wrap(idxB), xT=xT,
        planA=planA, planB=planB,
        offA=[int(v) for v in offA], offB=[int(v) for v in offB],
        groups=groups, SA=SA, SB=SB, RH=RH,
        row_of_node=outpos_of_node, table_row_of_node=row_of_node,
    )


def _plan_key(plans):
    return tuple((p["dmin"], p["S"], tuple(p["runs"])) for p in plans)


def _build(cfg, planA, planB, offA, offB, groups, SA, SB, RH):
    import sys
    if "/opt/trn_rl_repo" not in sys.path:
        sys.path.insert(0, "/opt/trn_rl_repo")
    import concourse.mybir as mybir
    import concourse.tile as tile
    from concourse import bacc
    from concourse.masks import make_identity

    f32 = mybir.dt.float32
    bf16 = mybir.dt.bfloat16
    R, GL = cfg["R"], cfg["GL"]
    F, HD = cfg["F"], cfg["H"]
    NPC = R * 128
    NTOT = 8 * NPC
    baseA, baseB = cfg["baseA"], cfg["baseB"]
    AF = HD + 2  # h | asrc | adst (f32 table-build layout)
    NG = len(groups)
    NPCa = RH * 128          # shard half sizes (round-major)
    NPCb = NPC - NPCa

    nc = bacc.Bacc("TRN2", target_bir_lowering=False, debug=False, num_devices=8)
    xT_t = nc.dram_tensor("xT", [F, NPC], f32, kind="ExternalInput")
    iA_t = nc.dram_tensor("idxA", [128, SA * 8], mybir.dt.int16, kind="ExternalInput")
    iB_t = nc.dram_tensor("idxB", [128, SB * 8], mybir.dt.int16, kind="ExternalInput")
    W1_t = nc.dram_tensor("W1", [F, HD], f32, kind="ExternalInput")
    W2_t = nc.dram_tensor("W2", [HD, HD], f32, kind="ExternalInput")
    av_t = nc.dram_tensor("avec", [4, HD], f32, kind="ExternalInput")
    bv_t = nc.dram_tensor("bvec", [2, HD], f32, kind="ExternalInput")
    out_t = nc.dram_tensor("out", [NPC, HD], f32, kind="ExternalOutput")

    # shard halves are separate tensors so the AllGather of half a never
    # false-depends on phase-A writes of half b
    shard1a = nc.dram_tensor("shard1a", [NPCa, 128], bf16, kind="Internal")
    shard2a = nc.dram_tensor("shard2a", [NPCa, 128], bf16, kind="Internal")
    if NPCb:
        shard1b = nc.dram_tensor("shard1b", [NPCb, 128], bf16, kind="Internal")
        shard2b = nc.dram_tensor("shard2b", [NPCb, 128], bf16, kind="Internal")
    else:
        shard1b = shard2b = None
    table1 = nc.dram_tensor("table1", [NTOT, 128], bf16, kind="Internal",
                            addr_space="Shared")
    table2 = nc.dram_tensor("table2", [NTOT, 128], bf16, kind="Internal",
                            addr_space="Shared")
    shards = {(1, 0): shard1a, (1, 1): shard1b, (2, 0): shard2a, (2, 1): shard2b}

    RG = [[0, 1, 2, 3, 4, 5, 6, 7]]

    with tile.TileContext(nc) as tc:
        with tc.tile_pool(name="const", bufs=1) as cp, \
             tc.tile_pool(name="spool", bufs=3) as sp, \
             tc.tile_pool(name="gpool", bufs=3) as gp, \
             tc.tile_pool(name="mpool", bufs=2) as mp, \
             tc.tile_pool(name="hpool", bufs=NG) as hp, \
             tc.tile_pool(name="psA", bufs=2, space="PSUM") as psA, \
             tc.tile_pool(name="psT", bufs=2, space="PSUM") as psT, \
             tc.tile_pool(name="psO", bufs=2, space="PSUM") as psO:

            ident = cp.tile([128, 128], f32)
            make_identity(nc, ident[:])
            identb = cp.tile([128, 128], bf16)
            make_identity(nc, identb[:])

            # weight prep: aug[l] = [W | W@a_src | W@a_dst]  ([K, AF])
            augs = []
            for l, (Wt, K) in enumerate(((W1_t, F), (W2_t, HD))):
                Wsb = cp.tile([K, HD], f32, tag=f"w{l}")
                nc.sync.dma_start(out=Wsb[:], in_=Wt.ap()[:, :])
                Wt_ps = psT.tile([HD, K], f32, tag="pst")
                nc.tensor.transpose(out=Wt_ps[:], in_=Wsb[:], identity=ident[:K, :K])
                Wtr = cp.tile([HD, K], f32, tag=f"wt{l}")
                nc.vector.tensor_copy(out=Wtr[:], in_=Wt_ps[:])
                aug = cp.tile([K, AF], f32, tag=f"aug{l}")
                nc.vector.tensor_copy(out=aug[:, 0:HD], in_=Wsb[:])
                for s in range(2):
                    acol = cp.tile([HD, 1], f32, tag=f"ac{l}{s}")
                    nc.sync.dma_start(
                        out=acol[:],
                        in_=av_t.ap()[2 * l + s:2 * l + s + 1, :].rearrange("a b -> b a"))
                    wa_ps = psT.tile([K, 1], f32, tag="pst")
                    nc.tensor.matmul(out=wa_ps[:], lhsT=Wtr[:], rhs=acol[:],
                                     start=True, stop=True)
                    nc.vector.tensor_copy(out=aug[:, HD + s:HD + s + 1], in_=wa_ps[:])
                augs.append(aug)

            # bias, replicated GL times: bbg[l] = [128, GL, HD]
            bbg = []
            for l in range(2):
                t = cp.tile([128, HD], f32, tag=f"b{l}")
                nc.sync.dma_start(out=t[:1, :], in_=bv_t.ap()[l:l + 1, :])
                nc.gpsimd.partition_broadcast(t[:], t[:1, :])
                tg = cp.tile([128, GL, HD], f32, tag=f"bg{l}")
                for rl in range(GL):
                    nc.vector.tensor_copy(out=tg[:, rl, :], in_=t[:])
                bbg.append(tg)

            # dummy row: h = 0, asrc = -1e30 (f32 at bf16 cols 64:66)
            dumrow = cp.tile([1, 128], bf16)
            nc.vector.memset(dumrow[:], 0.0)
            nc.vector.memset(dumrow[:, 64:66].bitcast(f32), -1e30)

            iA_sb = cp.tile([128, SA * 8], mybir.dt.int16)
            nc.sync.dma_start(out=iA_sb[:], in_=iA_t.ap()[:, :])
            iB_sb = cp.tile([128, SB * 8], mybir.dt.int16)
            nc.sync.dma_start(out=iB_sb[:], in_=iB_t.ap()[:, :])

            adst_own1 = cp.tile([128, R], f32, tag="adst1")
            adst_own2 = cp.tile([128, R], f32, tag="adst2")
            adst_own = [adst_own1, adst_own2]

            def table_chunk_write(t0, n, hs, lnum, layer):
                # hs: SBUF [AF, n*128] f-major -> bf16 node-major rows for
                # rounds t0..t0+n-1 (n*AF <= 512 so one PSUM bank holds the
                # batched transpose)
                htp = psT.tile([128, n * AF], f32, tag="pst",
                               padded_shape=[128, 512])
                for j in range(n):
                    nc.tensor.transpose(out=htp[:, j * AF:(j + 1) * AF],
                                        in_=hs[:, 128 * j:128 * (j + 1)],
                                        identity=ident[:AF, :AF])
                hv = htp[:, 0:n * AF].rearrange("p (n f) -> p n f", n=n)
                chunk = sp.tile([128, n, 128], bf16, tag="chunk",
                                padded_shape=[128, 8, 128])
                nc.vector.tensor_copy(out=chunk[:, 0:n, 0:HD],
                                      in_=hv[:, :, 0:HD])
                nc.vector.tensor_copy(
                    out=chunk[:, 0:n, 64:66].bitcast(f32),
                    in_=hv[:, :, HD:HD + 1])
                nc.vector.tensor_copy(
                    out=chunk[:, 0:n, 66:68].bitcast(f32),
                    in_=hv[:, :, HD + 1:HD + 2])
                nc.vector.tensor_copy(out=adst_own[layer][:, t0:t0 + n],
                                      in_=hv[:, :, HD + 1])
                t1 = t0 + n
                for (hf, r0, r1) in (((0, t0, min(t1, RH)),
                                      (1, max(t0, RH), t1))):
                    if r1 <= r0:
                        continue
                    base = 0 if hf == 0 else RH
                    nc.sync.dma_start(
                        out=shards[(lnum, hf)].ap()[
                            128 * (r0 - base):128 * (r1 - base), :].rearrange(
                            "(r p) f -> p r f", r=r1 - r0),
                        in_=chunk[:, r0 - t0:r1 - t0, :])

            def allgather_half(lnum, table, hf):
                n_in = NPCa if hf == 0 else NPCb
                if n_in == 0:
                    return
                shard = shards[(lnum, hf)]
                o0 = 0 if hf == 0 else 8 * NPCa
                nc.gpsimd.collective_compute(
                    "AllGather", mybir.AluOpType.bypass, RG,
                    ins=[shard.ap()[:, :]],
                    outs=[table.ap()[o0:o0 + 8 * n_in, :]])
                for base in (baseA, baseB):
                    if o0 <= base < o0 + 8 * n_in:
                        nc.gpsimd.dma_start(out=table.ap()[base:base + 1, :],
                                            in_=dumrow[:])

            def phase_A1():
                t0 = 0
                while t0 < R:
                    n = min(4, R - t0, RH - t0 if t0 < RH else R - t0)
                    rhs = sp.tile([F, n * 128], f32, tag="parhs",
                                  padded_shape=[F, 512])
                    nc.sync.dma_start(
                        out=rhs[:, 0:n * 128],
                        in_=xT_t.ap()[:, 128 * t0:128 * (t0 + n)])
                    hp_ = psA.tile([AF, n * 128], f32, tag="paps",
                                   padded_shape=[AF, 512])
                    nc.tensor.matmul(out=hp_[:, 0:n * 128], lhsT=augs[0][:],
                                     rhs=rhs[:, 0:n * 128],
                                     start=True, stop=True)
                    hs = sp.tile([AF, n * 128], f32, tag="pahs",
                                 padded_shape=[AF, 512])
                    nc.scalar.copy(out=hs[:, 0:n * 128], in_=hp_[:, 0:n * 128])
                    table_chunk_write(t0, n, hs, 1, 0)
                    t0 += n
                    if t0 == RH:
                        allgather_half(1, table1, 0)
                allgather_half(1, table1, 1)

            def phase_B_group(layer, g, table, adst):
                final = layer == 1
                g0, gl = groups[g]
                plA, plB = planA[g], planB[g]
                BA, BB = plA["S"], plB["S"]
                GA = gp.tile([128, max(BA, 1), 128], bf16, tag="G")
                if BA:
                    nc.gpsimd.dma_gather(
                        out_ap=GA[:, 0:BA, :], in_ap=table.ap()[baseA:, :],
                        idxs_ap=iA_sb[:, offA[g] * 8:offA[g + 1] * 8],
                        num_idxs=128 * BA, num_idxs_reg=128 * BA,
                        elem_size=128, single_packet=False)
                GB = gp.tile([128, max(BB, 1), 128], bf16, tag="G")
                if BB:
                    nc.gpsimd.dma_gather(
                        out_ap=GB[:, 0:BB, :], in_ap=table.ap()[baseB:, :],
                        idxs_ap=iB_sb[:, offB[g] * 8:offB[g + 1] * 8],
                        num_idxs=128 * BB, num_idxs_reg=128 * BB,
                        elem_size=128, single_packet=False)

                po = psO.tile([128, gl, HD + 1], f32, tag="po")
                nmm = sum(pl["dmin"] + len(pl["runs"])
                          for pl, B in ((plA, BA), (plB, BB)) if B)
                mm_i = 0
                for (G, pl, B, btag) in ((GA, plA, BA, "a"), (GB, plB, BB, "b")):
                    if B == 0:
                        continue
                    dmin, runs = pl["dmin"], pl["runs"]
                    # scores on the asrc (f32) subfield; adst replicated per
                    # round across the interleaved rect blocks + tail runs
                    arep = mp.tile([128, B, 1], f32, tag="arep" + btag)
                    if dmin:
                        ar4 = arep[:, 0:dmin * gl, :].rearrange(
                            "p (d r) o -> p d r o", d=dmin)
                        for rl in range(gl):
                            nc.vector.tensor_copy(
                                out=ar4[:, 0:dmin, rl, 0],
                                in_=adst[:, g0 + rl:g0 + rl + 1].to_broadcast(
                                    [128, dmin]))
                    for (d, rl0, n, blk0) in runs:
                        nc.vector.tensor_copy(
                            out=arep[:, blk0:blk0 + n, 0],
                            in_=adst[:, g0 + rl0:g0 + rl0 + n])
                    zt = mp.tile([128, B, 1], f32, tag="zt" + btag)
                    nc.vector.tensor_tensor(
                        out=zt[:, 0:B, :], in0=G[:, 0:B, 64:66].bitcast(f32),
                        in1=arep[:, 0:B, :], op=mybir.AluOpType.add)
                    z2 = mp.tile([128, B, 1], f32, tag="z2" + btag)
                    nc.vector.tensor_scalar(
                        out=z2[:, 0:B, :], in0=zt[:, 0:B, :],
                        scalar1=cfg["slope"], scalar2=None,
                        op0=mybir.AluOpType.mult)
                    lt = mp.tile([128, B, 1], f32, tag="lt" + btag)
                    nc.vector.tensor_tensor(
                        out=lt[:, 0:B, :], in0=zt[:, 0:B, :], in1=z2[:, 0:B, :],
                        op=mybir.AluOpType.max)
                    # t (bf16) overwrites the asrc-lo slot -> col 64
                    nc.scalar.activation(
                        out=G[:, 0:B, 64:65], in_=lt[:, 0:B, :],
                        func=mybir.ActivationFunctionType.Exp)
                    # weighted messages in place: h *= t
                    nc.vector.tensor_tensor(
                        out=G[:, 0:B, 0:HD], in0=G[:, 0:B, 0:HD],
                        in1=G[:, 0:B, 64:65].to_broadcast([128, B, HD]),
                        op=mybir.AluOpType.mult)
                    if dmin:
                        G4 = G[:, 0:dmin * gl, 0:HD + 1].rearrange(
                            "p (d r) f -> p d r f", d=dmin)
                        for d in range(dmin):
                            nc.tensor.matmul(
                                out=po[:, 0:gl, :], lhsT=identb[:],
                                rhs=G4[:, d, :, :],
                                start=mm_i == 0, stop=mm_i == nmm - 1)
                            mm_i += 1
                    for (d, rl0, n, blk0) in runs:
                        nc.tensor.matmul(
                            out=po[:, rl0:rl0 + n, :], lhsT=identb[:],
                            rhs=G[:, blk0:blk0 + n, 0:HD + 1],
                            start=mm_i == 0, stop=mm_i == nmm - 1,
                            skip_group_check=True)
                        mm_i += 1

                den = mp.tile([128, gl, 1], f32, tag="den")
                nc.vector.tensor_scalar_max(out=den[:, 0:gl, :],
                                            in0=po[:, 0:gl, HD:HD + 1],
                                            scalar1=1e-16)
                rd = mp.tile([128, gl, 1], f32, tag="rd")
                nc.vector.reciprocal(out=rd[:, 0:gl, :], in_=den[:, 0:gl, :])
                h = (mp if final else hp).tile([128, gl, HD], f32,
                                               tag="hfin" + str(layer))
                nc.vector.tensor_tensor(
                    out=h[:, 0:gl, :], in0=po[:, 0:gl, 0:HD],
                    in1=rd[:, 0:gl, :].to_broadcast([128, gl, HD]),
                    op=mybir.AluOpType.mult)
                nc.vector.tensor_tensor(out=h[:, 0:gl, :], in0=h[:, 0:gl, :],
                                        in1=bbg[layer][:, 0:gl, :],
                                        op=mybir.AluOpType.add)
                if final:
                    nc.sync.dma_start(
                        out=out_t.ap()[128 * g0:128 * (g0 + gl), :].rearrange(
                            "(r p) f -> p r f", r=gl),
                        in_=h[:, 0:gl, :])
                else:
                    nc.scalar.activation(out=h[:, 0:gl, :], in_=h[:, 0:gl, :],
                                         func=mybir.ActivationFunctionType.Relu)
                    hkeep.append(h)

            def phase_A2_group(g):
                g0, gl = groups[g]
                h = hkeep[g]
                ht7 = sp.tile([HD, gl * 128], f32, tag="hTs",
                              padded_shape=[HD, GL * 128])
                for rl in range(gl):
                    htr = psT.tile([HD, 128], f32, tag="pst2")
                    nc.tensor.transpose(out=htr[:], in_=h[:, rl, :],
                                        identity=ident[:])
                    nc.scalar.copy(out=ht7[:, 128 * rl:128 * (rl + 1)],
                                   in_=htr[:])
                done = 0
                while done < gl:
                    n = min(4, gl - done)
                    hp2 = psA.tile([AF, n * 128], f32, tag="paps",
                                   padded_shape=[AF, 512])
                    nc.tensor.matmul(out=hp2[:, 0:n * 128], lhsT=augs[1][:],
                                     rhs=ht7[:, 128 * done:128 * (done + n)],
                                     start=True, stop=True)
                    hs2 = sp.tile([AF, n * 128], f32, tag="pahs",
                                  padded_shape=[AF, 512])
                    nc.scalar.copy(out=hs2[:, 0:n * 128], in_=hp2[:, 0:n * 128])
                    table_chunk_write(g0 + done, n, hs2, 2, 1)
                    done += n

            hkeep = []
            phase_A1()
            ag2a_done = False
            for g in range(NG):
                phase_B_group(0, g, table1, adst_own[0])
                phase_A2_group(g)
                g0, gl = groups[g]
                if not ag2a_done and g0 + gl >= RH:
                    allgather_half(2, table2, 0)
                    ag2a_done = True
            allgather_half(2, table2, 1)
            for g in range(NG):
                phase_B_group(1, g, table2, adst_own[1])

    nc.compile()
    return nc


def _make_cfg(N, F, H):
    if N >= 32768:
        return dict(N=N, R=98, GL=7, baseA=32768, baseB=67585, span=32766,
                    F=F, H=H, slope=0.2)
    NTOT = max(2048, ((N + 128 + 1023) // 1024) * 1024)
    R = NTOT // 1024
    return dict(N=N, R=R, GL=min(7, R), baseA=NTOT // 4, baseB=(3 * NTOT) // 4,
                span=min(32766, (5 * NTOT) // 8), F=F, H=H, slope=0.2)


def _make_in_maps(inputs, prep):
    avec = np.stack([np.asarray(inputs["a1_src"]), np.asarray(inputs["a1_dst"]),
                     np.asarray(inputs["a2_src"]), np.asarray(inputs["a2_dst"])]
                    ).astype(np.float32)
    bvec = np.stack([np.asarray(inputs["b1"]), np.asarray(inputs["b2"])]
                    ).astype(np.float32)
    in_maps = []
    for k in range(8):
        in_maps.append({
            "xT": prep["xT"][k], "idxA": prep["idxA"][k], "idxB": prep["idxB"][k],
            "W1": np.asarray(inputs["W1"], dtype=np.float32),
            "W2": np.asarray(inputs["W2"], dtype=np.float32),
            "avec": avec, "bvec": bvec,
        })
    return in_maps


def kernel(x, edge_index, W1, a1_src, a1_dst, b1, W2, a2_src, a2_dst, b2):
    import sys
    if "/opt/trn_rl_repo" not in sys.path:
        sys.path.insert(0, "/opt/trn_rl_repo")
    from concourse import bass_utils

    x = np.asarray(x)
    cfg = _make_cfg(x.shape[0], x.shape[1], np.asarray(W1).shape[1])
    prep = _host_prep(x, edge_index, cfg)
    key = (cfg["N"], cfg["R"], prep["SA"], prep["SB"],
           _plan_key(prep["planA"]), _plan_key(prep["planB"]))
    if key not in _CACHE:
        _CACHE[key] = _build(cfg, prep["planA"], prep["planB"],
                             prep["offA"], prep["offB"], prep["groups"],
                             prep["SA"], prep["SB"], prep["RH"])
    nc = _CACHE[key]

    in_maps = _make_in_maps(
        dict(a1_src=a1_src, a1_dst=a1_dst, a2_src=a2_src, a2_dst=a2_dst,
             b1=b1, b2=b2, W1=W1, W2=W2), prep)
    res = bass_utils.run_bass_kernel_spmd(nc, in_maps, core_ids=list(range(8)))
    shards = np.concatenate([res.results[k]["out"] for k in range(8)], axis=0)
    return shards[prep["row_of_node"]].astype(np.float32)


# revision 31
# speedup vs baseline: 58.8464x; 1.0024x over previous
"""Two-layer GAT on 8 Trainium2 NeuronCores.

Strategy (dst-sharded, node-major bf16 table):
 - Nodes are degree-sorted into 128-node blocks; blocks are dealt round-robin
   to the 8 cores so every core runs an identical static schedule. Rounds are
   grouped GL=7 at a time; one PSUM bank accumulates all 7 rounds (7*65=455
   cols <= 512).
 - Per layer, each core computes its shard of a node table
   [row: h(64 bf16) | asrc(f32) | adst(f32) | pad -> 128 bf16 = 256B], an
   AllGather replicates the full table, then each (group, bank) issues ONE
   dma_gather whose index order interleaves the group's rounds
   (block = slot_d * gl + round_local), so slot-d of all 7 rounds is one
   matmul rhs [128, gl, 65] accumulating into po[128, gl, 65].
 - t_e = exp(leaky_relu(asrc_src + adst_dst)) is computed on the gathered
   chunk in a handful of wide vector ops; t overwrites the asrc slot (bf16)
   so rhs cols 0:65 are [t*h | t] after one broadcast multiply.
 - Softmax max-subtraction is algebraically a no-op here (scores are O(10)).
   Padded slots gather a dummy table row with asrc = -1e30 so t == 0.
 - Two int16-index banks (A at row baseA, B at baseB) cover the >64K rows.
"""
import numpy as np

_CACHE = {}


def _host_prep(x, edge_index, cfg):
    N, C, R, GL = cfg["N"], 8, cfg["R"], cfg["GL"]
    NPC = R * 128            # rows per core shard
    NTOT = C * NPC
    # rounds in table half 0 (AllGather split point); RH == R disables split
    # (shared-output AllGather makes the split a wash on HW, and fewer/bigger
    # collectives are strictly better in the cost model)
    RH = (R + 1) // 2 if cfg.get("agsplit", False) else R
    baseA, baseB, span = cfg["baseA"], cfg["baseB"], cfg["span"]
    A_hi = min(NTOT - 1, baseA + span)
    B_lo = max(0, baseB - span)
    assert A_hi >= B_lo - 1

    groups = [(i * GL, min(GL, R - i * GL)) for i in range((R + GL - 1) // GL)]
    NG = len(groups)

    src = np.asarray(edge_index[0], dtype=np.int64)
    dst = np.asarray(edge_index[1], dtype=np.int64)
    E = src.shape[0]

    deg = np.bincount(dst, minlength=N)
    odeg = np.bincount(src, minlength=N)
    order = np.argsort(-deg, kind="stable")
    all_nodes = np.concatenate([order, np.full(NTOT - N, -1, dtype=np.int64)])

    m = np.arange(NTOT)
    b = m // 128
    p = m % 128
    rnd = b // C
    core = b % C
    # half-major row layout so each AllGather half is a contiguous table range
    half = rnd // RH
    rw = rnd % RH
    row_of_listpos = (half * (C * RH * 128) + core * (RH * 128) + 128 * rw + p)

    # within each round, put the highest OUT-degree nodes on rows inside the
    # flex window [B_lo, A_hi] — their out-edges become bank-flexible, which
    # shrinks the forced-bank count tails that drive slot padding
    for r in range(R):
        sel = np.flatnonzero(rnd == r)
        rows = row_of_listpos[sel]
        flex = (rows >= B_lo) & (rows <= A_hi)
        nd = all_nodes[sel]
        od = np.where(nd >= 0, odeg[np.clip(nd, 0, None)], -1)
        pos_order = np.argsort(~flex, kind="stable")   # flex positions first
        nd_order = np.argsort(-od, kind="stable")      # high out-degree first
        newnd = np.empty_like(nd)
        newnd[pos_order] = nd[nd_order]
        all_nodes[sel] = newnd

    # bank holes + dummy rows must hold pad nodes
    special_rows = {baseA - 1, baseA, baseB - 1, baseB}
    row_to_listpos = np.empty(NTOT, dtype=np.int64)
    row_to_listpos[row_of_listpos] = m
    pad_positions = [i for i in range(NTOT - 1, -1, -1) if all_nodes[i] < 0]
    pi = 0
    for r in special_rows:
        lp = row_to_listpos[r]
        if all_nodes[lp] >= 0:
            while pi < len(pad_positions):
                q = pad_positions[pi]; pi += 1
                if row_of_listpos[q] not in special_rows and all_nodes[q] < 0:
                    all_nodes[lp], all_nodes[q] = all_nodes[q], all_nodes[lp]
                    break

    node_at_listpos = all_nodes
    row_of_node = np.full(N, -1, dtype=np.int64)
    real = node_at_listpos >= 0
    row_of_node[node_at_listpos[real]] = row_of_listpos[real]

    sr = row_of_node[src]
    dr = row_of_node[dst]

    rnd_of_node_row = np.empty(NTOT, dtype=np.int64)
    rnd_of_node_row[row_of_listpos] = rnd

    # bank per edge (0=A, 1=B); per-group thresholds TA/TB minimize the
    # rectangular slot count max(cntA) + max(cntB) over the group
    canA = sr <= A_hi
    canB = sr >= B_lo
    forcedA = canA & ~canB
    forcedB = ~canA & canB
    flex = canA & canB
    nA0 = np.bincount(dr[forcedA], minlength=NTOT)
    nB0 = np.bincount(dr[forcedB], minlength=NTOT)
    nf = np.bincount(dr[flex], minlength=NTOT)
    degr = np.bincount(dr, minlength=NTOT)

    # per-round thresholds TA/TB minimize the per-round slot count
    # max(cntA) + max(cntB) (jagged layout bills each round individually)
    A0r_ = np.zeros(R, dtype=np.int64)
    B0r_ = np.zeros(R, dtype=np.int64)
    Mr_ = np.zeros(R, dtype=np.int64)
    np.maximum.at(A0r_, rnd_of_node_row, nA0)
    np.maximum.at(B0r_, rnd_of_node_row, nB0)
    np.maximum.at(Mr_, rnd_of_node_row, degr)
    costr = np.maximum(Mr_, A0r_ + B0r_)
    TAr = np.clip((costr + 1) // 2, A0r_, costr - B0r_)
    TBr = costr - TAr
    r_of_row = rnd_of_node_row
    lo = np.maximum(nA0, degr - TBr[r_of_row])
    hi = np.minimum(TAr[r_of_row], nA0 + nf)
    cntA = np.clip((degr + 1) // 2, lo, hi)

    o = np.argsort(dr[flex], kind="stable")
    flex_idx = np.nonzero(flex)[0][o]
    grp = dr[flex_idx]
    uniq, first = np.unique(grp, return_index=True)
    fr = np.arange(len(grp)) - first[np.searchsorted(uniq, grp)]
    bank = np.ones(E, dtype=np.int8)
    bank[forcedA] = 0
    bank[flex_idx] = (fr >= (cntA[grp] - nA0[grp])).astype(np.int8)
    cntB = degr - cntA

    DAr = np.zeros(R, dtype=np.int64)
    DBr = np.zeros(R, dtype=np.int64)
    np.maximum.at(DAr, rnd_of_node_row, cntA)
    np.maximum.at(DBr, rnd_of_node_row, cntB)
    # rect-A slot d=0 must exist in every round: the group's first matmul
    # (bank A, d=0) covers all po columns with start=True
    DAr = np.maximum(DAr, 1)

    # slot position within (dst, bank); negative gather indices first so the
    # final slot of each (dst, bank) sequence is non-negative where possible
    idxval = np.where(bank == 0, sr - baseA, sr - baseB)
    nonneg = (idxval >= 0).astype(np.int8)
    o2 = np.lexsort((nonneg, bank, dr))
    grp2 = dr[o2] * 2 + bank[o2]
    uniq2, first2 = np.unique(grp2, return_index=True)
    dpos = np.arange(E) - first2[np.searchsorted(uniq2, grp2)]
    d_of_edge = np.empty(E, dtype=np.int64)
    d_of_edge[o2] = dpos

    cnt_nonneg_A = np.bincount(dr[(bank == 0) & (idxval >= 0)], minlength=NTOT)
    cnt_nonneg_B = np.bincount(dr[(bank == 1) & (idxval >= 0)], minlength=NTOT)
    p_of_row = np.empty(NTOT, dtype=np.int64)
    p_of_row[row_of_listpos] = p
    is_last_p = p_of_row == 127

    def build_plan(Dr, cnt, cnt_nonneg):
        # jagged layout per (group, bank): rect core (d < dmin over the
        # group's rounds) + per-d tail runs of rounds still alive. Returns
        # (plans, offsets); bumps Dr where the call's final slot would be a
        # full all-negative (dst,bank) sequence (HW drops trailing negatives).
        while True:
            plans = []
            off = [0]
            redo = False
            for g, (g0, gl) in enumerate(groups):
                D = Dr[g0:g0 + gl]
                dmin = int(D.min())
                S = dmin * gl
                runs = []
                last_rl = gl - 1
                for d in range(dmin, int(D.max())):
                    rl = 0
                    while rl < gl:
                        if D[rl] > d:
                            rl0 = rl
                            while rl < gl and D[rl] > d:
                                rl += 1
                            runs.append((d, rl0, rl - rl0, S))
                            S += rl - rl0
                            last_rl = rl - 1
                        else:
                            rl += 1
                plans.append(dict(dmin=dmin, S=S, runs=runs))
                off.append(off[-1] + S)
                # guard: dst at (round of final block, p=127) must not have a
                # full all-negative slot sequence
                rr = g0 + last_rl
                sel = is_last_p & (rnd_of_node_row == rr)
                if np.any(sel & (cnt == Dr[rr]) & (cnt > 0) & (cnt_nonneg == 0)):
                    Dr[rr] += 1
                    redo = True
                    break
            if not redo:
                return plans, off

    planA, offA = build_plan(DAr, cntA, cnt_nonneg_A)
    planB, offB = build_plan(DBr, cntB, cnt_nonneg_B)
    SA, SB = int(offA[-1]), int(offB[-1])

    def blk_lut(Dr, plans, off):
        maxD = max(int(Dr.max()), 1)
        lut = np.full((R, maxD), -1, dtype=np.int64)
        for g, (g0, gl) in enumerate(groups):
            pl = plans[g]
            dmin = pl["dmin"]
            for rl in range(gl):
                for d in range(dmin):
                    lut[g0 + rl, d] = off[g] + d * gl + rl
            for (d, rl0, n, blk0) in pl["runs"]:
                for j in range(n):
                    lut[g0 + rl0 + j, d] = off[g] + blk0 + j
        return lut

    lutA = blk_lut(DAr, planA, offA)
    lutB = blk_lut(DBr, planB, offB)

    idxA = np.zeros((C, SA * 128), dtype=np.int32)
    idxB = np.zeros((C, SB * 128), dtype=np.int32)
    e_half = dr // (C * RH * 128)
    e_rem = dr % (C * RH * 128)
    e_core = e_rem // (RH * 128)
    e_rnd = e_half * RH + (e_rem % (RH * 128)) // 128
    e_p = dr % 128
    isA = bank == 0
    blkA = lutA[e_rnd[isA], d_of_edge[isA]]
    assert (blkA >= 0).all()
    idxA[e_core[isA], blkA * 128 + e_p[isA]] = sr[isA] - baseA
    isB = ~isA
    blkB = lutB[e_rnd[isB], d_of_edge[isB]]
    assert (blkB >= 0).all()
    idxB[e_core[isB], blkB * 128 + e_p[isB]] = sr[isB] - baseB
    assert idxA.min() >= -32768 and idxA.max() <= 32766
    assert idxB.min() >= -32768 and idxB.max() <= 32766
    # final slot of every call must be non-negative (trailing negatives drop)
    for g in range(NG):
        if offA[g + 1] > offA[g]:
            assert (idxA[:, offA[g + 1] * 128 - 1] >= 0).all()
        if offB[g + 1] > offB[g]:
            assert (idxB[:, offB[g + 1] * 128 - 1] >= 0).all()

    def wrap(a):  # [C, S*128] -> [C, 128, S*8] int16 (16-wrap, replicated x8)
        Cn, tot = a.shape
        if tot == 0:
            return np.zeros((Cn, 128, 0), dtype=np.int16)
        w = a.reshape(Cn, tot // 16, 16).transpose(0, 2, 1)
        return np.ascontiguousarray(np.tile(w, (1, 8, 1))).astype(np.int16)

    # shard-local position (round-major) differs from the half-major table row
    shardpos = 128 * rnd + p
    xT = np.zeros((C, x.shape[1], NPC), dtype=np.float32)
    xf = np.asarray(x, dtype=np.float32)
    for k in range(C):
        sel = (core == k) & real
        xT[k][:, shardpos[sel]] = xf[node_at_listpos[sel]].T

    # host-side gather index: node -> (core, shardpos) in concatenated output
    outpos_of_listpos = core * NPC + shardpos
    outpos_of_node = np.full(N, -1, dtype=np.int64)
    outpos_of_node[node_at_listpos[real]] = outpos_of_listpos[real]

    return dict(
        idxA=wrap(idxA), idxB=wrap(idxB), xT=xT,
        planA=planA, planB=planB,
        offA=[int(v) for v in offA], offB=[int(v) for v in offB],
        groups=groups, SA=SA, SB=SB, RH=RH,
        row_of_node=outpos_of_node, table_row_of_node=row_of_node,
    )


def _plan_key(plans):
    return tuple((p["dmin"], p["S"], tuple(p["runs"])) for p in plans)


def _build(cfg, planA, planB, offA, offB, groups, SA, SB, RH):
    import sys
    if "/opt/trn_rl_repo" not in sys.path:
        sys.path.insert(0, "/opt/trn_rl_repo")
    import concourse.mybir as mybir
    import concourse.tile as tile
    from concourse import bacc
    from concourse.masks import make_identity

    f32 = mybir.dt.float32
    bf16 = mybir.dt.bfloat16
    R, GL = cfg["R"], cfg["GL"]
    F, HD = cfg["F"], cfg["H"]
    NPC = R * 128
    NTOT = 8 * NPC
    baseA, baseB = cfg["baseA"], cfg["baseB"]
    AF = HD + 2  # h | asrc | adst (f32 table-build layout)
    NG = len(groups)
    NPCa = RH * 128          # shard half sizes (round-major)
    NPCb = NPC - NPCa

    nc = bacc.Bacc("TRN2", target_bir_lowering=False, debug=False, num_devices=8)
    xT_t = nc.dram_tensor("xT", [F, NPC], f32, kind="ExternalInput")
    iA_t = nc.dram_tensor("idxA", [128, SA * 8], mybir.dt.int16, kind="ExternalInput")
    iB_t = nc.dram_tensor("idxB", [128, SB * 8], mybir.dt.int16, kind="ExternalInput")
    W1_t = nc.dram_tensor("W1", [F, HD], f32, kind="ExternalInput")
    W2_t = nc.dram_tensor("W2", [HD, HD], f32, kind="ExternalInput")
    av_t = nc.dram_tensor("avec", [4, HD], f32, kind="ExternalInput")
    bv_t = nc.dram_tensor("bvec", [2, HD], f32, kind="ExternalInput")
    out_t = nc.dram_tensor("out", [NPC, HD], f32, kind="ExternalOutput")

    # shard halves are separate tensors so the AllGather of half a never
    # false-depends on phase-A writes of half b
    shard1a = nc.dram_tensor("shard1a", [NPCa, 128], bf16, kind="Internal")
    shard2a = nc.dram_tensor("shard2a", [NPCa, 128], bf16, kind="Internal")
    if NPCb:
        shard1b = nc.dram_tensor("shard1b", [NPCb, 128], bf16, kind="Internal")
        shard2b = nc.dram_tensor("shard2b", [NPCb, 128], bf16, kind="Internal")
    else:
        shard1b = shard2b = None
    table1 = nc.dram_tensor("table1", [NTOT, 128], bf16, kind="Internal",
                            addr_space="Shared")
    table2 = nc.dram_tensor("table2", [NTOT, 128], bf16, kind="Internal",
                            addr_space="Shared")
    shards = {(1, 0): shard1a, (1, 1): shard1b, (2, 0): shard2a, (2, 1): shard2b}

    RG = [[0, 1, 2, 3, 4, 5, 6, 7]]

    with tile.TileContext(nc) as tc:
        with tc.tile_pool(name="const", bufs=1) as cp, \
             tc.tile_pool(name="spool", bufs=3) as sp, \
             tc.tile_pool(name="gpool", bufs=3) as gp, \
             tc.tile_pool(name="mpool", bufs=2) as mp, \
             tc.tile_pool(name="hpool", bufs=NG) as hp, \
             tc.tile_pool(name="psA", bufs=2, space="PSUM") as psA, \
             tc.tile_pool(name="psT", bufs=2, space="PSUM") as psT, \
             tc.tile_pool(name="psO", bufs=2, space="PSUM") as psO:

            ident = cp.tile([128, 128], f32)
            make_identity(nc, ident[:])
            identb = cp.tile([128, 128], bf16)
            make_identity(nc, identb[:])

            # weight prep: aug[l] = [W | W@a_src | W@a_dst]  ([K, AF])
            augs = []
            for l, (Wt, K) in enumerate(((W1_t, F), (W2_t, HD))):
                Wsb = cp.tile([K, HD], f32, tag=f"w{l}")
                nc.sync.dma_start(out=Wsb[:], in_=Wt.ap()[:, :])
                Wt_ps = psT.tile([HD, K], f32, tag="pst")
                nc.tensor.transpose(out=Wt_ps[:], in_=Wsb[:], identity=ident[:K, :K])
                Wtr = cp.tile([HD, K], f32, tag=f"wt{l}")
                nc.vector.tensor_copy(out=Wtr[:], in_=Wt_ps[:])
                aug = cp.tile([K, AF], f32, tag=f"aug{l}")
                nc.vector.tensor_copy(out=aug[:, 0:HD], in_=Wsb[:])
                for s in range(2):
                    acol = cp.tile([HD, 1], f32, tag=f"ac{l}{s}")
                    nc.sync.dma_start(
                        out=acol[:],
                        in_=av_t.ap()[2 * l + s:2 * l + s + 1, :].rearrange("a b -> b a"))
                    wa_ps = psT.tile([K, 1], f32, tag="pst")
                    nc.tensor.matmul(out=wa_ps[:], lhsT=Wtr[:], rhs=acol[:],
                                     start=True, stop=True)
                    nc.vector.tensor_copy(out=aug[:, HD + s:HD + s + 1], in_=wa_ps[:])
                augs.append(aug)

            # bias, replicated GL times: bbg[l] = [128, GL, HD]
            bbg = []
            for l in range(2):
                t = cp.tile([128, HD], f32, tag=f"b{l}")
                nc.sync.dma_start(out=t[:1, :], in_=bv_t.ap()[l:l + 1, :])
                nc.gpsimd.partition_broadcast(t[:], t[:1, :])
                tg = cp.tile([128, GL, HD], f32, tag=f"bg{l}")
                for rl in range(GL):
                    nc.vector.tensor_copy(out=tg[:, rl, :], in_=t[:])
                bbg.append(tg)

            # dummy row: h = 0, asrc = -1e30 (f32 at bf16 cols 64:66)
            dumrow = cp.tile([1, 128], bf16)
            nc.vector.memset(dumrow[:], 0.0)
            nc.vector.memset(dumrow[:, 64:66].bitcast(f32), -1e30)

            iA_sb = cp.tile([128, SA * 8], mybir.dt.int16)
            nc.sync.dma_start(out=iA_sb[:], in_=iA_t.ap()[:, :])
            iB_sb = cp.tile([128, SB * 8], mybir.dt.int16)
            nc.sync.dma_start(out=iB_sb[:], in_=iB_t.ap()[:, :])

            adst_own1 = cp.tile([128, R], f32, tag="adst1")
            adst_own2 = cp.tile([128, R], f32, tag="adst2")
            adst_own = [adst_own1, adst_own2]

            def table_chunk_write(t0, n, hs, lnum, layer):
                # hs: SBUF [AF, n*128] f-major -> bf16 node-major rows for
                # rounds t0..t0+n-1 (n*AF <= 512 so one PSUM bank holds the
                # batched transpose)
                htp = psT.tile([128, n * AF], f32, tag="pst",
                               padded_shape=[128, 512])
                for j in range(n):
                    nc.tensor.transpose(out=htp[:, j * AF:(j + 1) * AF],
                                        in_=hs[:, 128 * j:128 * (j + 1)],
                                        identity=ident[:AF, :AF])
                hv = htp[:, 0:n * AF].rearrange("p (n f) -> p n f", n=n)
                chunk = sp.tile([128, n, 128], bf16, tag="chunk",
                                padded_shape=[128, 8, 128])
                nc.vector.tensor_copy(out=chunk[:, 0:n, 0:HD],
                                      in_=hv[:, :, 0:HD])
                nc.vector.tensor_copy(
                    out=chunk[:, 0:n, 64:66].bitcast(f32),
                    in_=hv[:, :, HD:HD + 1])
                nc.vector.tensor_copy(
                    out=chunk[:, 0:n, 66:68].bitcast(f32),
                    in_=hv[:, :, HD + 1:HD + 2])
                nc.vector.tensor_copy(out=adst_own[layer][:, t0:t0 + n],
                                      in_=hv[:, :, HD + 1])
                t1 = t0 + n
                for (hf, r0, r1) in (((0, t0, min(t1, RH)),
                                      (1, max(t0, RH), t1))):
                    if r1 <= r0:
                        continue
                    base = 0 if hf == 0 else RH
                    nc.sync.dma_start(
                        out=shards[(lnum, hf)].ap()[
                            128 * (r0 - base):128 * (r1 - base), :].rearrange(
                            "(r p) f -> p r f", r=r1 - r0),
                        in_=chunk[:, r0 - t0:r1 - t0, :])

            def allgather_half(lnum, table, hf):
                n_in = NPCa if hf == 0 else NPCb
                if n_in == 0:
                    return
                shard = shards[(lnum, hf)]
                o0 = 0 if hf == 0 else 8 * NPCa
                nc.gpsimd.collective_compute(
                    "AllGather", mybir.AluOpType.bypass, RG,
                    ins=[shard.ap()[:, :]],
                    outs=[table.ap()[o0:o0 + 8 * n_in, :]])
                for base in (baseA, baseB):
                    if o0 <= base < o0 + 8 * n_in:
                        nc.gpsimd.dma_start(out=table.ap()[base:base + 1, :],
                                            in_=dumrow[:])

            def phase_A1():
                t0 = 0
                while t0 < R:
                    n = min(4, R - t0, RH - t0 if t0 < RH else R - t0)
                    rhs = sp.tile([F, n * 128], f32, tag="parhs",
                                  padded_shape=[F, 512])
                    nc.sync.dma_start(
                        out=rhs[:, 0:n * 128],
                        in_=xT_t.ap()[:, 128 * t0:128 * (t0 + n)])
                    hp_ = psA.tile([AF, n * 128], f32, tag="paps",
                                   padded_shape=[AF, 512])
                    nc.tensor.matmul(out=hp_[:, 0:n * 128], lhsT=augs[0][:],
                                     rhs=rhs[:, 0:n * 128],
                                     start=True, stop=True)
                    hs = sp.tile([AF, n * 128], f32, tag="pahs",
                                 padded_shape=[AF, 512])
                    nc.scalar.copy(out=hs[:, 0:n * 128], in_=hp_[:, 0:n * 128])
                    table_chunk_write(t0, n, hs, 1, 0)
                    t0 += n
                    if t0 == RH:
                        allgather_half(1, table1, 0)
                allgather_half(1, table1, 1)

            def phase_B_group(layer, g, table, adst):
                final = layer == 1
                g0, gl = groups[g]
                plA, plB = planA[g], planB[g]
                BA, BB = plA["S"], plB["S"]
                GA = gp.tile([128, max(BA, 1), 128], bf16, tag="G")
                if BA:
                    nc.gpsimd.dma_gather(
                        out_ap=GA[:, 0:BA, :], in_ap=table.ap()[baseA:, :],
                        idxs_ap=iA_sb[:, offA[g] * 8:offA[g + 1] * 8],
                        num_idxs=128 * BA, num_idxs_reg=128 * BA,
                        elem_size=128, single_packet=False)
                GB = gp.tile([128, max(BB, 1), 128], bf16, tag="G")
                if BB:
                    nc.gpsimd.dma_gather(
                        out_ap=GB[:, 0:BB, :], in_ap=table.ap()[baseB:, :],
                        idxs_ap=iB_sb[:, offB[g] * 8:offB[g + 1] * 8],
                        num_idxs=128 * BB, num_idxs_reg=128 * BB,
                        elem_size=128, single_packet=False)

                po = psO.tile([128, gl, HD + 1], f32, tag="po")
                nmm = sum(pl["dmin"] + len(pl["runs"])
                          for pl, B in ((plA, BA), (plB, BB)) if B)
                mm_i = 0
                for (G, pl, B, btag) in ((GA, plA, BA, "a"), (GB, plB, BB, "b")):
                    if B == 0:
                        continue
                    dmin, runs = pl["dmin"], pl["runs"]
                    # scores on the asrc (f32) subfield; adst replicated per
                    # round across the interleaved rect blocks + tail runs
                    arep = mp.tile([128, B, 1], f32, tag="arep" + btag)
                    if dmin:
                        ar4 = arep[:, 0:dmin * gl, :].rearrange(
                            "p (d r) o -> p d r o", d=dmin)
                        for rl in range(gl):
                            nc.vector.tensor_copy(
                                out=ar4[:, 0:dmin, rl, 0],
                                in_=adst[:, g0 + rl:g0 + rl + 1].to_broadcast(
                                    [128, dmin]))
                    for (d, rl0, n, blk0) in runs:
                        nc.vector.tensor_copy(
                            out=arep[:, blk0:blk0 + n, 0],
                            in_=adst[:, g0 + rl0:g0 + rl0 + n])
                    zt = mp.tile([128, B, 1], f32, tag="zt" + btag)
                    nc.vector.tensor_tensor(
                        out=zt[:, 0:B, :], in0=G[:, 0:B, 64:66].bitcast(f32),
                        in1=arep[:, 0:B, :], op=mybir.AluOpType.add)
                    z2 = mp.tile([128, B, 1], f32, tag="z2" + btag)
                    nc.vector.tensor_scalar(
                        out=z2[:, 0:B, :], in0=zt[:, 0:B, :],
                        scalar1=cfg["slope"], scalar2=None,
                        op0=mybir.AluOpType.mult)
                    lt = mp.tile([128, B, 1], f32, tag="lt" + btag)
                    nc.vector.tensor_tensor(
                        out=lt[:, 0:B, :], in0=zt[:, 0:B, :], in1=z2[:, 0:B, :],
                        op=mybir.AluOpType.max)
                    # t (bf16) overwrites the asrc-lo slot -> col 64
                    nc.scalar.activation(
                        out=G[:, 0:B, 64:65], in_=lt[:, 0:B, :],
                        func=mybir.ActivationFunctionType.Exp)
                    # weighted messages in place: h *= t
                    nc.vector.tensor_tensor(
                        out=G[:, 0:B, 0:HD], in0=G[:, 0:B, 0:HD],
                        in1=G[:, 0:B, 64:65].to_broadcast([128, B, HD]),
                        op=mybir.AluOpType.mult)
                    if dmin:
                        G4 = G[:, 0:dmin * gl, 0:HD + 1].rearrange(
                            "p (d r) f -> p d r f", d=dmin)
                        for d in range(dmin):
                            nc.tensor.matmul(
                                out=po[:, 0:gl, :], lhsT=identb[:],
                                rhs=G4[:, d, :, :],
                                start=mm_i == 0, stop=mm_i == nmm - 1)
                            mm_i += 1
                    for (d, rl0, n, blk0) in runs:
                        nc.tensor.matmul(
                            out=po[:, rl0:rl0 + n, :], lhsT=identb[:],
                            rhs=G[:, blk0:blk0 + n, 0:HD + 1],
                            start=mm_i == 0, stop=mm_i == nmm - 1,
                            skip_group_check=True)
                        mm_i += 1

                den = mp.tile([128, gl, 1], f32, tag="den")
                nc.vector.tensor_scalar_max(out=den[:, 0:gl, :],
                                            in0=po[:, 0:gl, HD:HD + 1],
                                            scalar1=1e-16)
                rd = mp.tile([128, gl, 1], f32, tag="rd")
                nc.vector.reciprocal(out=rd[:, 0:gl, :], in_=den[:, 0:gl, :])
                h = (mp if final else hp).tile([128, gl, HD], f32,
                                               tag="hfin" + str(layer))
                nc.vector.tensor_tensor(
                    out=h[:, 0:gl, :], in0=po[:, 0:gl, 0:HD],
                    in1=rd[:, 0:gl, :].to_broadcast([128, gl, HD]),
                    op=mybir.AluOpType.mult)
                nc.vector.tensor_tensor(out=h[:, 0:gl, :], in0=h[:, 0:gl, :],
                                        in1=bbg[layer][:, 0:gl, :],
                                        op=mybir.AluOpType.add)
                if final:
                    nc.sync.dma_start(
                        out=out_t.ap()[128 * g0:128 * (g0 + gl), :].rearrange(
                            "(r p) f -> p r f", r=gl),
                        in_=h[:, 0:gl, :])
                else:
                    nc.scalar.activation(out=h[:, 0:gl, :], in_=h[:, 0:gl, :],
                                         func=mybir.ActivationFunctionType.Relu)
                    hkeep[g] = h

            def phase_A2_group(g):
                g0, gl = groups[g]
                h = hkeep[g]
                ht7 = sp.tile([HD, gl * 128], f32, tag="hTs",
                              padded_shape=[HD, GL * 128])
                for rl in range(gl):
                    htr = psT.tile([HD, 128], f32, tag="pst2")
                    nc.tensor.transpose(out=htr[:], in_=h[:, rl, :],
                                        identity=ident[:])
                    nc.scalar.copy(out=ht7[:, 128 * rl:128 * (rl + 1)],
                                   in_=htr[:])
                done = 0
                while done < gl:
                    n = min(4, gl - done)
                    hp2 = psA.tile([AF, n * 128], f32, tag="paps",
                                   padded_shape=[AF, 512])
                    nc.tensor.matmul(out=hp2[:, 0:n * 128], lhsT=augs[1][:],
                                     rhs=ht7[:, 128 * done:128 * (done + n)],
                                     start=True, stop=True)
                    hs2 = sp.tile([AF, n * 128], f32, tag="pahs",
                                  padded_shape=[AF, 512])
                    nc.scalar.copy(out=hs2[:, 0:n * 128], in_=hp2[:, 0:n * 128])
                    table_chunk_write(g0 + done, n, hs2, 2, 1)
                    done += n

            # group processing order: smallest group first (fast pipeline fill
            # right after the AllGather), second-smallest last (short drain
            # tail). Only when the AllGather is unsplit — the split needs
            # round-prefix completion order.
            if RH >= R and NG > 2:
                size_of = [planA[g]["S"] + planB[g]["S"] for g in range(NG)]
                srt = sorted(range(NG), key=lambda g: size_of[g])
                g_order = [srt[0]] + [g for g in range(NG)
                                      if g not in (srt[0], srt[1])] + [srt[1]]
            else:
                g_order = list(range(NG))

            hkeep = {}
            phase_A1()
            ag2a_done = False
            for g in g_order:
                phase_B_group(0, g, table1, adst_own[0])
                phase_A2_group(g)
                if not ag2a_done and RH < R \
                        and g_order == list(range(NG)) \
                        and groups[g][0] + groups[g][1] >= RH:
                    allgather_half(2, table2, 0)
                    ag2a_done = True
            if not ag2a_done:
                allgather_half(2, table2, 0)
            allgather_half(2, table2, 1)
            for g in g_order:
                phase_B_group(1, g, table2, adst_own[1])

    nc.compile()
    return nc


def _make_cfg(N, F, H):
    if N >= 32768:
        return dict(N=N, R=98, GL=7, baseA=32768, baseB=67585, span=32766,
                    F=F, H=H, slope=0.2)
    NTOT = max(2048, ((N + 128 + 1023) // 1024) * 1024)
    R = NTOT // 1024
    return dict(N=N, R=R, GL=min(7, R), baseA=NTOT // 4, baseB=(3 * NTOT) // 4,
                span=min(32766, (5 * NTOT) // 8), F=F, H=H, slope=0.2)


def _make_in_maps(inputs, prep):
    avec = np.stack([np.asarray(inputs["a1_src"]), np.asarray(inputs["a1_dst"]),
                     np.asarray(inputs["a2_src"]), np.asarray(inputs["a2_dst"])]
                    ).astype(np.float32)
    bvec = np.stack([np.asarray(inputs["b1"]), np.asarray(inputs["b2"])]
                    ).astype(np.float32)
    in_maps = []
    for k in range(8):
        in_maps.append({
            "xT": prep["xT"][k], "idxA": prep["idxA"][k], "idxB": prep["idxB"][k],
            "W1": np.asarray(inputs["W1"], dtype=np.float32),
            "W2": np.asarray(inputs["W2"], dtype=np.float32),
            "avec": avec, "bvec": bvec,
        })
    return in_maps


def kernel(x, edge_index, W1, a1_src, a1_dst, b1, W2, a2_src, a2_dst, b2):
    import sys
    if "/opt/trn_rl_repo" not in sys.path:
        sys.path.insert(0, "/opt/trn_rl_repo")
    from concourse import bass_utils

    x = np.asarray(x)
    cfg = _make_cfg(x.shape[0], x.shape[1], np.asarray(W1).shape[1])
    prep = _host_prep(x, edge_index, cfg)
    key = (cfg["N"], cfg["R"], prep["SA"], prep["SB"],
           _plan_key(prep["planA"]), _plan_key(prep["planB"]))
    if key not in _CACHE:
        _CACHE[key] = _build(cfg, prep["planA"], prep["planB"],
                             prep["offA"], prep["offB"], prep["groups"],
                             prep["SA"], prep["SB"], prep["RH"])
    nc = _CACHE[key]

    in_maps = _make_in_maps(
        dict(a1_src=a1_src, a1_dst=a1_dst, a2_src=a2_src, a2_dst=a2_dst,
             b1=b1, b2=b2, W1=W1, W2=W2), prep)
    res = bass_utils.run_bass_kernel_spmd(nc, in_maps, core_ids=list(range(8)))
    shards = np.concatenate([res.results[k]["out"] for k in range(8)], axis=0)
    return shards[prep["row_of_node"]].astype(np.float32)
